# revision 44
# baseline (speedup 1.0000x reference)
"""Trainium2 Bass kernel for nn_AcceleratedAttention (GQA transformer block:
fused QKV proj -> causal GQA attention -> out proj), tensor-parallel over
heads on 8 NeuronCores.

Sharding (per core c of 8):
  - q heads {c, c+8, c+16, c+24} and kv head c (all four q heads share kv
    head c, so K/V are computed once per core).
  - Per 512-row s-block: QKV projection, then attention for ALL FOUR heads
    interleaved (QK pairs -> exp on Act -> AV chains pumped with lag), so
    the Activation engine load is spread uniformly and PE stays the
    bottleneck everywhere.
  - Attention runs with scores transposed ([kv, q] tiles).  The AV product
    uses the probability tile as the STATIONARY operand:
    out[q, hd] = pt[kv, q].T @ [V | 1], a 65-column moving matmul at full
    128-row contraction; the ones column delivers the softmax row-sum in
    psum column 64 (normalization = per-partition reciprocal+scale on DVE).
  - ONE merged AllToAll per batch ([2048 rows, 256 chans]; 256-row chunks)
    — the collective cost model charges a flat ~15us per collective, so
    fewer+earlier collectives win.  Core c ends up owning rows
    [256c, 256c+256) of each batch.
  - Epilogue per batch: received [row, chan] blocks are PE-transposed to
    [chan, row]; out-proj = 8 chunks of 16 accumulating matmuls (full
    2048-contraction in one psum bank, no fp16 partial accumulate).
    W_out tiles are preloaded during the last attention blocks.

All matmuls run in fp16 (1 cycle/row on PE; psum fp32).
"""
import sys
from collections import deque

sys.path.insert(0, "/opt/trn_rl_repo")

import numpy as np

DIM = 2048
N_HEADS = 32
N_KV = 8
HD = 64
BATCH = 2
SEQ = 2048
NCORE = 8
S_TOT = BATCH * SEQ          # 4096
S_SLICE = S_TOT // NCORE     # 512
NKC = DIM // 128             # 16 contraction tiles
NSB = S_TOT // 512           # 8 s-blocks for qkv proj
NT = SEQ // 128              # 16 kv tiles per batch
NQB = SEQ // 512             # 4 sq blocks per batch
SCALE = 0.125                # 1/sqrt(64)

_CACHE = {}
TRACE_SIM = False


def _build(causal: bool):
    import concourse.mybir as mybir
    import concourse.tile as tile
    from concourse import bacc
    from concourse.masks import make_identity

    F32 = mybir.dt.float32
    F16 = mybir.dt.float16
    AF = mybir.ActivationFunctionType
    MUL = mybir.AluOpType.mult
    ADD = mybir.AluOpType.add

    nc = bacc.Bacc()
    xT = nc.declare_dram_parameter("xT", [DIM, S_TOT], F16, isOutput=False)
    wq = nc.declare_dram_parameter("wq", [DIM, 384], F16, isOutput=False)
    wo = nc.declare_dram_parameter("wo", [DIM, DIM], F16, isOutput=False)
    ind = nc.declare_dram_parameter("ind", [128, 128], F16, isOutput=False)
    # rows [0,256) = batch0 rows [256c,256c+256), rows [256,512) = batch1
    out = nc.declare_dram_parameter("out", [S_SLICE, DIM], F32, isOutput=True)

    with tile.TileContext(nc, trace_sim=TRACE_SIM) as tc:
        with (
            tc.tile_pool(name="consts", bufs=1) as consts,
            tc.tile_pool(name="qkvp", bufs=1) as qkvp,
            tc.tile_pool(name="vextp", bufs=32) as vextp,
            tc.tile_pool(name="ptp", bufs=17) as ptp,
            tc.tile_pool(name="rsp", bufs=4) as rsp,
            tc.tile_pool(name="ostp", bufs=5) as ostp,
            tc.tile_pool(name="otp", bufs=1) as otp,
            tc.tile_pool(name="aotp", bufs=2) as aotp,
            tc.tile_pool(name="outstp", bufs=3) as outstp,
            tc.tile_pool(name="partp", bufs=8) as partp,
            tc.tile_pool(name="wopA", bufs=2) as wopA,
            tc.tile_pool(name="ps_qkv", bufs=2, space="PSUM") as ps_qkv,
            tc.tile_pool(name="ps_st", bufs=2, space="PSUM") as ps_st,
            tc.tile_pool(name="ps_av", bufs=2, space="PSUM") as ps_av,
            tc.tile_pool(name="dram", bufs=1, space="DRAM") as dram,
        ):
            # ---- constants
            idn = consts.tile([128, 128], F16)
            make_identity(nc, idn[:])
            ind_t = consts.tile([128, 128], F16)
            nc.sync.dma_start(ind_t[:], ind[:])

            # Merged AllToAll for batch 0 ([2048 rows, 256 chans]); batch 1
            # split by head pair into two [2048, 128] collectives so the
            # first pair's out-proj half covers the last collective's
            # window.  256-row chunks -> core c gets rows [256c, 256c+256).
            a2a_in = {}
            a2a_out = {}
            a2a_in[0] = dram.tile([SEQ, 256], F16, name="a2ain0")
            a2a_out[0] = dram.tile([SEQ, 256], F16, name="a2aout0")
            for pr in range(2):
                a2a_in[(1, pr)] = dram.tile([SEQ, 128], F16,
                                            name=f"a2ain1p{pr}")
                a2a_out[(1, pr)] = dram.tile([SEQ, 128], F16,
                                             name=f"a2aout1p{pr}")

            xT_r = xT.rearrange("(nk p) s -> p nk s", p=128)
            wo_r = wo.rearrange("(nk p) e -> p nk e", p=128)

            # deferred AV chains, pumped with ~4-item lag behind exp
            pend_a = deque()

            def pump(q, drain_to=0):
                while len(q) > drain_to:
                    q.popleft()()

            # ---- qkv tiles
            qkv_t = {(m, sb): qkvp.tile([128, 512], F16, tag=f"qkv{m}_{sb}",
                                        name=f"qkv{m}_{sb}")
                     for m in range(3) for sb in range(NSB)}
            # K replicated at partitions 64:128 so odd heads' Q (upper
            # partition half) sees a matching base partition.
            kv2_t = {sb: qkvp.tile([128, 512], F16, tag=f"kv2_{sb}",
                                   name=f"kv2_{sb}")
                     for sb in range(NSB)}

            def q_ap(h, b, sqb, c0, c1):
                t = qkv_t[(h // 2, b * NQB + sqb)]
                base = 64 * (h % 2)
                return t[base:base + 64, c0:c1]

            def k_ap(h, b, j):
                sb, off = divmod(j, 4)
                if h % 2 == 0:
                    return qkv_t[(2, b * NQB + sb)][0:64,
                                                    off * 128:(off + 1) * 128]
                return kv2_t[b * NQB + sb][64:128, off * 128:(off + 1) * 128]

            vexts = {}

            # ---- phase 1 (QKV projection), one 512-row s-block at a time
            with (
                tc.tile_pool(name="wqp", bufs=1) as wqp,
                tc.tile_pool(name="xtp", bufs=2) as xtp,
            ):
                wq_t = wqp.tile([128, NKC, 384], F16)
                wq_r = wq.rearrange("(nk p) m -> p nk m", p=128)
                nc.sync.dma_start(wq_t[:, 0:1], wq_r[:, 0:1])
                nc.sync.dma_start(wq_t[:, 1:3], wq_r[:, 1:3])
                nc.sync.dma_start(wq_t[:, 3:8], wq_r[:, 3:8])
                nc.sync.dma_start(wq_t[:, 8:NKC], wq_r[:, 8:NKC])

                # phase-1 work for a block is split into single-op closures
                # so it can be drip-fed between attention pairs of the
                # PREVIOUS block (fills PE during Act-bound stretches and
                # keeps the p-state hot).
                pend_p1 = deque()

                def phase1_closures(sb):
                    xt = xtp.tile([128, NKC, 512], F16, tag="xt",
                                  name=f"xt{sb}")
                    sl = slice(sb * 512, (sb + 1) * 512)
                    if sb == 0:
                        # split across two idle DMA rings at startup
                        nc.gpsimd.dma_start(xt[:, 0:1], xT_r[:, 0:1, sl])
                        nc.scalar.dma_start(xt[:, 8:10], xT_r[:, 8:10, sl])
                        nc.gpsimd.dma_start(xt[:, 1:2], xT_r[:, 1:2, sl])
                        nc.gpsimd.dma_start(xt[:, 2:4], xT_r[:, 2:4, sl])
                        nc.scalar.dma_start(xt[:, 10:NKC], xT_r[:, 10:NKC, sl])
                        nc.gpsimd.dma_start(xt[:, 4:8], xT_r[:, 4:8, sl])
                    else:
                        nc.gpsimd.dma_start(xt[:], xT_r[:, :, sl])
                    ops = []
                    for m in (2, 0, 1):  # kv first: feeds vext early
                        cell = {}

                        def mk_mm(m, k, cell):
                            def mm():
                                if k == 0:
                                    cell["acc"] = ps_qkv.tile(
                                        [128, 512], F32, tag="qkv",
                                        name=f"qkvacc{sb}_{m}")
                                nc.tensor.matmul(
                                    cell["acc"][:],
                                    wq_t[:, k, m * 128:(m + 1) * 128],
                                    xt[:, k, :],
                                    start=(k == 0),
                                    stop=(k == NKC - 1),
                                )
                            return mm

                        for k in range(NKC):
                            ops.append(mk_mm(m, k, cell))

                        def mk_copy(m, cell):
                            def cp():
                                nc.vector.tensor_copy(
                                    qkv_t[(m, sb)][:], cell["acc"][:])
                                if m == 2:
                                    nc.sync.dma_start(
                                        kv2_t[sb][64:128, :],
                                        qkv_t[(2, sb)][0:64, :])
                            return cp

                        ops.append(mk_copy(m, cell))

                    b, sqb = divmod(sb, NQB)
                    for jo in range(4):
                        j = sqb * 4 + jo
                        vexts[(b, j)] = vextp.tile(
                            [128, 65], F16, tag="vext", name=f"vx_{b}_{j}")

                        def mk_vext(jo, j):
                            def vext():
                                vx = vexts[(b, j)]
                                tp = ps_st.tile([128, 64], F16, tag="st",
                                                name=f"vt_{b}_{j}")
                                nc.tensor.transpose(
                                    tp[:],
                                    qkv_t[(2, sb)][64:128,
                                                   jo * 128:(jo + 1) * 128],
                                    idn[64:128, 64:128],
                                )
                                nc.vector.tensor_copy(vx[:, 0:64], tp[:])
                                nc.vector.memset(vx[:, 64:65], 1.0)
                            return vext

                        ops.append(mk_vext(jo, j))
                    return ops

                # ---- attention machinery
                ost_tiles = {}
                ost_count = {}

                def get_ost(h, b):
                    if (h, b) not in ost_tiles:
                        ost_tiles[(h, b)] = ostp.tile(
                            [128, 16, HD], F16, tag="ost", name=f"ost{h}_{b}")
                        ost_count[(h, b)] = 0
                    return ost_tiles[(h, b)]

                def finish_ost(h, b, half=None):
                    # half=0: rows qt 0:8 eagerly; half=1: qt 8:16 (tile
                    # freed).  Splitting lets the collective start sooner
                    # after the last chain.
                    if half == 0:
                        ost = ost_tiles[(h, b)]
                    else:
                        ost = ost_tiles.pop((h, b))
                    buf = a2a_in[0] if b == 0 else a2a_in[(1, h // 2)]
                    co = 64 * h if b == 0 else 64 * (h % 2)
                    dst = buf.rearrange("(j q2 p) c -> p j q2 c",
                                        p=128, q2=2)
                    if half == 0:
                        nc.sync.dma_start(dst[:, 0:4, :, co:co + 64],
                                          ost[:, 0:8])
                    else:
                        nc.sync.dma_start(dst[:, 4:8, :, co:co + 64],
                                          ost[:, 8:16])

                def make_chain(h, b, sqb, qco, pts, offs):
                    qt = 4 * sqb + qco
                    jhi = qt if causal else NT - 1

                    def chain():
                        av = ps_av.tile([128, 65], F32, tag="av",
                                        name=f"av{h}_{b}_{qt}")
                        for j in range(jhi + 1):
                            c0 = qco * 128 - offs[j]
                            nc.tensor.matmul(
                                av[:],
                                pts[j][:, c0:c0 + 128],
                                vexts[(b, j)][:],
                                start=(j == 0),
                                stop=(j == jhi),
                                skip_group_check=True,
                            )
                        rsr = rsp.tile([128, 1], F32, tag="rsr",
                                       name=f"rsr{h}_{b}_{qt}")
                        nc.vector.reciprocal_approx_fast(rsr[:], av[:, 64:65])
                        ost = get_ost(h, b)
                        nc.vector.tensor_scalar_mul(
                            ost[:, qt, :], av[:, 0:64], rsr[:])
                        ost_count[(h, b)] += 1
                        if ost_count[(h, b)] == 8:
                            finish_ost(h, b, half=0)
                        elif ost_count[(h, b)] == 16:
                            finish_ost(h, b, half=1)

                    return chain

                def attn_block(h, b, sqb, tmod=2, last=False):
                    jmax = 4 * sqb + 3 if causal else NT - 1
                    ndiag0 = 4 * sqb if causal else jmax + 1
                    pts = {}
                    offs = {}
                    get_ost(h, b)
                    npair = (jmax + 1) // 2

                    def qk_pair(gi):
                        stp = ps_st.tile([128, 2, 512], F32, tag="st",
                                         name=f"st{h}_{b}_{sqb}_{gi}")
                        for idx in range(2):
                            j = 2 * gi + idx
                            off = (max(0, j * 128 - sqb * 512)
                                   if causal else 0)
                            n = 512 - off
                            nc.tensor.matmul(
                                stp[:, idx, :n],
                                k_ap(h, b, j),
                                q_ap(h, b, sqb, off, 512),
                                start=True,
                                stop=True,
                            )
                            offs[j] = off
                        return stp

                    def exp_pair(gi, stp):
                        j0 = 2 * gi
                        pt2 = ptp.tile([128, 2, 512], F16, tag="pt",
                                       name=f"pt{h}_{b}_{sqb}_{gi}")
                        if j0 + 1 < ndiag0:
                            # both halves unmasked: one fused activation
                            nc.scalar.activation(
                                pt2[:], stp[:], AF.Exp, scale=SCALE)
                        else:
                            for idx in range(2):
                                j = j0 + idx
                                n = 512 - offs[j]
                                nc.scalar.activation(
                                    pt2[:, idx, :n], stp[:, idx, :n],
                                    AF.Exp, scale=SCALE)
                                if causal and j >= ndiag0:
                                    nc.vector.tensor_tensor(
                                        pt2[:, idx, 0:128],
                                        pt2[:, idx, 0:128], ind_t[:], MUL)
                        pts[j0] = pt2[:, 0, :]
                        pts[j0 + 1] = pt2[:, 1, :]

                    stps = {0: qk_pair(0)}
                    for gi in range(npair):
                        if gi + 1 < npair:
                            stps[gi + 1] = qk_pair(gi + 1)
                        exp_pair(gi, stps.pop(gi))
                        for idx in range(2):
                            j = 2 * gi + idx
                            pump(pend_a, 4)
                            for _ in range(2):
                                if pend_p1:
                                    pend_p1.popleft()()
                            if (not pend_p1 and pend_tail
                                    and j % tmod == 0
                                    and not (last and gi >= npair - 3)):
                                pend_tail.popleft()()
                            if causal and j >= ndiag0:
                                pend_a.append(
                                    make_chain(h, b, sqb, j - ndiag0,
                                               pts, offs))
                    if not causal:
                        for qco in range(4):
                            pend_a.append(
                                make_chain(h, b, sqb, qco, pts, offs))
                        pump(pend_a, 4)

                def emit_a2a(key):
                    nc.gpsimd.collective_compute(
                        "AllToAll",
                        mybir.AluOpType.bypass,
                        replica_groups=[list(range(NCORE))],
                        ins=[a2a_in[key].opt()],
                        outs=[a2a_out[key].opt()],
                    )

                # ================= emission =================
                wot = {}

                def load_wo(eb, pool, queue):
                    w = pool.tile([128, NKC, 512], F16, tag="wo",
                                  name=f"wo{eb}")
                    queue.dma_start(
                        w[:], wo_r[:, :, eb * 512:(eb + 1) * 512])
                    wot[eb] = w

                ot_tiles = {}
                pend_tail = deque()

                def emit_ot(b):
                    ot = otp.tile([128, 2, 8, 256], F16, tag="ot",
                                  name=f"ot{b}")
                    src = a2a_out[b].rearrange("(j q2 p) c -> p q2 j c",
                                               p=128, q2=2)
                    nc.gpsimd.dma_start(ot[:, 0], src[:, 0])
                    nc.gpsimd.dma_start(ot[:, 1], src[:, 1])
                    aoT = aotp.tile([128, 2, 16, 128], F16, tag="aot",
                                    name=f"aoT{b}")
                    ot_tiles[b] = (ot, aoT)

                def emit_oth(pr):
                    # batch-1 pair buffer: chans [0:128] of kt = 2j + pr
                    ot = otp.tile([128, 2, 8, 128], F16, tag="oth",
                                  name=f"ot1p{pr}")
                    src = a2a_out[(1, pr)].rearrange(
                        "(j q2 p) c -> p q2 j c", p=128, q2=2)
                    nc.gpsimd.dma_start(ot[:, 0], src[:, 0])
                    nc.scalar.dma_start(ot[:, 1], src[:, 1])
                    if 1 not in ot_tiles:
                        # [p, ss, pair, j, c]: kt = 2j + pair
                        aoT = aotp.tile([128, 2, 2, 8, 128], F16, tag="aot",
                                        name="aoT1")
                        ot_tiles[1] = (None, aoT)
                    return ot

                def tp4(b, ss, gg):
                    # 4 PE transposes into one psum tile + one DVE copy
                    def f():
                        ot, aoT = ot_tiles[b]
                        tpt = ps_st.tile([128, 4, 128], F16, tag="st",
                                         name=f"tp{b}_{ss}_{gg}")
                        for i in range(4):
                            kt = gg * 4 + i
                            j, ch = divmod(kt, 2)
                            nc.tensor.transpose(
                                tpt[:, i, :],
                                ot[:, ss, j, ch * 128:(ch + 1) * 128],
                                idn[:],
                            )
                        nc.vector.tensor_copy(
                            aoT[:, ss, gg * 4:gg * 4 + 4, :], tpt[:])
                    return f

                def tp4h(oth, pr, ss, jg):
                    # 4 transposes of b1 pair pr, source j in [4jg, 4jg+4)
                    def f():
                        _, aoT = ot_tiles[1]
                        tpt = ps_st.tile([128, 4, 128], F16, tag="st",
                                         name=f"tph{pr}_{ss}_{jg}")
                        for i in range(4):
                            j = jg * 4 + i
                            nc.tensor.transpose(
                                tpt[:, i, :], oth[:, ss, j, :], idn[:])
                        nc.vector.tensor_copy(
                            aoT[:, ss, pr, jg * 4:jg * 4 + 4, :], tpt[:])
                    return f

                parts = {}

                def half_chunk(eb, ss, pr):
                    # b1 out-proj over kt = pr, pr+2, ..., pr+14.  Pair 0
                    # parks the partial in fp16 SBUF; pair 1 adds it back.
                    def f():
                        _, aoT = ot_tiles[1]
                        ops = ps_qkv.tile([128, 512], F32, tag="qkv",
                                          name=f"opsh{eb}_{ss}_{pr}")
                        for jj in range(8):
                            nc.tensor.matmul(
                                ops[:],
                                aoT[:, ss, pr, jj, :],
                                wot[eb][:, 2 * jj + pr, :],
                                start=(jj == 0),
                                stop=(jj == 7),
                            )
                        if pr == 0:
                            part = partp.tile([128, 512], F16, tag="part",
                                              name=f"part{eb}_{ss}")
                            nc.vector.tensor_copy(part[:], ops[:])
                            parts[(eb, ss)] = part
                        else:
                            og = outstp.tile([128, 512], F32, tag="out",
                                             name=f"ogh{eb}_{ss}")
                            nc.vector.tensor_tensor(
                                og[:], parts[(eb, ss)][:], ops[:], ADD)
                            nc.sync.dma_start(
                                out[256 + ss * 128:256 + ss * 128 + 128,
                                    eb * 512:(eb + 1) * 512],
                                og[:],
                            )
                    return f

                def chunk_quarters(b, eb, ss):
                    # out-proj chunk split into 4 pumpable quarters
                    cell = {}

                    def mk_q(q):
                        def f():
                            _, aoT = ot_tiles[b]
                            if q == 0:
                                cell["ops"] = ps_qkv.tile(
                                    [128, 512], F32, tag="qkv",
                                    name=f"ops{b}_{eb}_{ss}")
                            for k in range(4 * q, 4 * q + 4):
                                nc.tensor.matmul(
                                    cell["ops"][:],
                                    aoT[:, ss, k, :],
                                    wot[eb][:, k, :],
                                    start=(k == 0),
                                    stop=(k == NKC - 1),
                                )
                            if q == 3:
                                og = outstp.tile([128, 512], F32, tag="out",
                                                 name=f"og{b}_{eb}_{ss}")
                                nc.vector.tensor_copy(og[:], cell["ops"][:])
                                nc.sync.dma_start(
                                    out[b * 256 + ss * 128:
                                        b * 256 + ss * 128 + 128,
                                        eb * 512:(eb + 1) * 512],
                                    og[:],
                                )
                        return f

                    return [mk_q(q) for q in range(4)]

                for sb in range(NSB):
                    if sb == 0:
                        for op in phase1_closures(0):
                            op()
                    pump(pend_p1)  # leftover phase-1 ops of this block
                    if sb + 1 < NSB:
                        pend_p1.extend(phase1_closures(sb + 1))
                    b, sqb = divmod(sb, NQB)
                    for h in (range(4) if b == 0 else (0, 1, 2)):
                        attn_block(h, b, sqb)
                        if sb == NSB - 1 and h == 1:
                            # heads 0,1 of batch 1 complete -> pair-0
                            # collective overlaps h2/h3 tail attention
                            pump(pend_a)
                            emit_a2a((1, 0))
                    if sb == NQB - 1:
                        # batch 0 attention fully emitted; chains drain in
                        # the pump lag -> collective as soon as osts land
                        pump(pend_a)
                        emit_a2a(0)
                        emit_ot(0)
                        load_wo(0, wopA, nc.sync)
                        load_wo(1, wopA, nc.sync)
                        # batch-0 epilogue work for the preloaded k-halves
                        # becomes PE filler during batch-1 attention
                        for ss in range(2):
                            for gg in range(4):
                                pend_tail.append(tp4(0, ss, gg))
                            for eb in (0, 1):
                                pend_tail.extend(chunk_quarters(0, eb, ss))
            # ---- batch-1 head 3 + epilogue.  Pair-0's collective runs
            # during this attention; its out-proj half covers pair-1's.
            # Unpumped b0 quarters become the reserve for the final window.
            with tc.tile_pool(name="wopB", bufs=2) as wopB:
                load_wo(2, wopB, nc.sync)
                load_wo(3, wopB, nc.sync)
                for ss in range(2):
                    for eb in (2, 3):
                        pend_tail.extend(chunk_quarters(0, eb, ss))
                # throttled pumping: keep most of the b0 reserve for the
                # pair-1 collective window
                for sqb in range(NQB):
                    attn_block(3, 1, sqb, tmod=6, last=(sqb == NQB - 1))
                pump(pend_a)
                emit_a2a((1, 1))
                pump(pend_tail)

                # pair-0 epilogue half (fills pair-1's collective window)
                ot_p0 = emit_oth(0)
                for ss in range(2):
                    for jg in range(2):
                        tp4h(ot_p0, 0, ss, jg)()
                for ss in range(2):
                    for eb in range(4):
                        half_chunk(eb, ss, 0)()
                # pair-1 epilogue half
                ot_p1 = emit_oth(1)
                for ss in range(2):
                    for jg in range(2):
                        tp4h(ot_p1, 1, ss, jg)()
                for ss in range(2):
                    for eb in range(4):
                        half_chunk(eb, ss, 1)()

    nc.compile()
    return nc


def _program(causal: bool):
    if causal not in _CACHE:
        _CACHE[causal] = _build(causal)
    return _CACHE[causal]


def _prep_in_maps(x, W_qkv, W_out):
    xT = x.reshape(S_TOT, DIM).T.astype(np.float16)  # [c, s] contiguous
    ind_np = (np.arange(128)[None, :] >= np.arange(128)[:, None]).astype(np.float16)

    # W_out cols permuted to the received-block channel order: block j
    # (256 chans) carries core j's heads {j, j+8, j+16, j+24}.
    wt = W_out.T.reshape(N_HEADS, HD, DIM)
    woT = np.ascontiguousarray(np.concatenate(
        [wt[[j, j + 8, j + 16, j + 24]].reshape(4 * HD, DIM)
         for j in range(NCORE)]
    )).astype(np.float16)

    in_maps = []
    for c in range(NCORE):
        rows = np.concatenate(
            [np.arange(h * HD, (h + 1) * HD) for h in (c, c + 8, c + 16, c + 24)]
            + [np.arange(2048 + c * HD, 2048 + (c + 1) * HD),
               np.arange(2560 + c * HD, 2560 + (c + 1) * HD)]
        )
        wq_shard = np.ascontiguousarray(W_qkv[rows].T).astype(np.float16)
        in_maps.append({"xT": xT, "wq": wq_shard, "wo": woT, "ind": ind_np})
    return in_maps


def kernel(**inputs) -> np.ndarray:
    from concourse.bass_utils import run_bass_kernel_spmd

    x = np.asarray(inputs["x"], np.float32)
    W_qkv = np.asarray(inputs["W_qkv"], np.float32)
    W_out = np.asarray(inputs["W_out"], np.float32)
    mask = np.asarray(inputs["mask"], np.float32)

    m2 = mask.reshape(SEQ, SEQ)
    iu = np.triu_indices(SEQ, 1)
    causal = bool((m2[iu] <= -1e8).all()) and bool((np.tril(m2) == 0.0).all())
    dense = bool((m2 == 0.0).all())
    if not (causal or dense):
        raise NotImplementedError("only causal or all-zero masks supported")

    nc = _program(causal)
    in_maps = _prep_in_maps(x, W_qkv, W_out)
    res = run_bass_kernel_spmd(nc, in_maps, list(range(NCORE))).results
    # core c owns rows [256c, 256c+256) of each batch
    full = np.empty((S_TOT, DIM), np.float32)
    for c in range(NCORE):
        o = res[c]["out"]
        full[256 * c:256 * c + 256] = o[0:256]
        full[SEQ + 256 * c:SEQ + 256 * c + 256] = o[256:512]
    return np.ascontiguousarray(full.reshape(BATCH, SEQ, DIM), dtype=np.float32)


# revision 45
# speedup vs baseline: 1.0000x; 1.0000x over previous
"""Trainium2 Bass kernel for nn_AcceleratedAttention (GQA transformer block:
fused QKV proj -> causal GQA attention -> out proj), tensor-parallel over
heads on 8 NeuronCores.

Sharding (per core c of 8):
  - q heads {c, c+8, c+16, c+24} and kv head c (all four q heads share kv
    head c, so K/V are computed once per core).
  - Per 512-row s-block: QKV projection, then attention for ALL FOUR heads
    interleaved (QK pairs -> exp on Act -> AV chains pumped with lag), so
    the Activation engine load is spread uniformly and PE stays the
    bottleneck everywhere.
  - Attention runs with scores transposed ([kv, q] tiles).  The AV product
    uses the probability tile as the STATIONARY operand:
    out[q, hd] = pt[kv, q].T @ [V | 1], a 65-column moving matmul at full
    128-row contraction; the ones column delivers the softmax row-sum in
    psum column 64 (normalization = per-partition reciprocal+scale on DVE).
  - ONE merged AllToAll per batch ([2048 rows, 256 chans]; 256-row chunks)
    — the collective cost model charges a flat ~15us per collective, so
    fewer+earlier collectives win.  Core c ends up owning rows
    [256c, 256c+256) of each batch.
  - Epilogue per batch: received [row, chan] blocks are PE-transposed to
    [chan, row]; out-proj = 8 chunks of 16 accumulating matmuls (full
    2048-contraction in one psum bank, no fp16 partial accumulate).
    W_out tiles are preloaded during the last attention blocks.

All matmuls run in fp16 (1 cycle/row on PE; psum fp32).
"""
import sys
from collections import deque

sys.path.insert(0, "/opt/trn_rl_repo")

import numpy as np

DIM = 2048
N_HEADS = 32
N_KV = 8
HD = 64
BATCH = 2
SEQ = 2048
NCORE = 8
S_TOT = BATCH * SEQ          # 4096
S_SLICE = S_TOT // NCORE     # 512
NKC = DIM // 128             # 16 contraction tiles
NSB = S_TOT // 512           # 8 s-blocks for qkv proj
NT = SEQ // 128              # 16 kv tiles per batch
NQB = SEQ // 512             # 4 sq blocks per batch
SCALE = 0.125                # 1/sqrt(64)

_CACHE = {}
TRACE_SIM = False


def _build(causal: bool):
    import concourse.mybir as mybir
    import concourse.tile as tile
    from concourse import bacc
    from concourse.masks import make_identity

    F32 = mybir.dt.float32
    F16 = mybir.dt.float16
    AF = mybir.ActivationFunctionType
    MUL = mybir.AluOpType.mult
    ADD = mybir.AluOpType.add

    nc = bacc.Bacc()
    xT = nc.declare_dram_parameter("xT", [DIM, S_TOT], F16, isOutput=False)
    wq = nc.declare_dram_parameter("wq", [DIM, 384], F16, isOutput=False)
    wo = nc.declare_dram_parameter("wo", [DIM, DIM], F16, isOutput=False)
    ind = nc.declare_dram_parameter("ind", [128, 128], F16, isOutput=False)
    # rows [0,256) = batch0 rows [256c,256c+256), rows [256,512) = batch1
    out = nc.declare_dram_parameter("out", [S_SLICE, DIM], F32, isOutput=True)

    with tile.TileContext(nc, trace_sim=TRACE_SIM) as tc:
        with (
            tc.tile_pool(name="consts", bufs=1) as consts,
            tc.tile_pool(name="qkvp", bufs=1) as qkvp,
            tc.tile_pool(name="vextp", bufs=32) as vextp,
            tc.tile_pool(name="ptp", bufs=17) as ptp,
            tc.tile_pool(name="rsp", bufs=4) as rsp,
            tc.tile_pool(name="ostp", bufs=5) as ostp,
            tc.tile_pool(name="otp", bufs=1) as otp,
            tc.tile_pool(name="aotp", bufs=2) as aotp,
            tc.tile_pool(name="outstp", bufs=3) as outstp,
            tc.tile_pool(name="partp", bufs=8) as partp,
            tc.tile_pool(name="wopA", bufs=2) as wopA,
            tc.tile_pool(name="ps_qkv", bufs=2, space="PSUM") as ps_qkv,
            tc.tile_pool(name="ps_st", bufs=2, space="PSUM") as ps_st,
            tc.tile_pool(name="ps_av", bufs=2, space="PSUM") as ps_av,
            tc.tile_pool(name="dram", bufs=1, space="DRAM") as dram,
        ):
            # ---- constants
            idn = consts.tile([128, 128], F16)
            make_identity(nc, idn[:])
            ind_t = consts.tile([128, 128], F16)
            nc.sync.dma_start(ind_t[:], ind[:])

            # Merged AllToAll for batch 0 ([2048 rows, 256 chans]); batch 1
            # split by head pair into two [2048, 128] collectives so the
            # first pair's out-proj half covers the last collective's
            # window.  256-row chunks -> core c gets rows [256c, 256c+256).
            a2a_in = {}
            a2a_out = {}
            a2a_in[0] = dram.tile([SEQ, 256], F16, name="a2ain0")
            a2a_out[0] = dram.tile([SEQ, 256], F16, name="a2aout0")
            for pr in range(2):
                a2a_in[(1, pr)] = dram.tile([SEQ, 128], F16,
                                            name=f"a2ain1p{pr}")
                a2a_out[(1, pr)] = dram.tile([SEQ, 128], F16,
                                             name=f"a2aout1p{pr}")

            xT_r = xT.rearrange("(nk p) s -> p nk s", p=128)
            wo_r = wo.rearrange("(nk p) e -> p nk e", p=128)

            # deferred AV chains, pumped with ~4-item lag behind exp
            pend_a = deque()

            def pump(q, drain_to=0):
                while len(q) > drain_to:
                    q.popleft()()

            # ---- qkv tiles
            qkv_t = {(m, sb): qkvp.tile([128, 512], F16, tag=f"qkv{m}_{sb}",
                                        name=f"qkv{m}_{sb}")
                     for m in range(3) for sb in range(NSB)}
            # K replicated at partitions 64:128 so odd heads' Q (upper
            # partition half) sees a matching base partition.
            kv2_t = {sb: qkvp.tile([128, 512], F16, tag=f"kv2_{sb}",
                                   name=f"kv2_{sb}")
                     for sb in range(NSB)}

            def q_ap(h, b, sqb, c0, c1):
                t = qkv_t[(h // 2, b * NQB + sqb)]
                base = 64 * (h % 2)
                return t[base:base + 64, c0:c1]

            def k_ap(h, b, j):
                sb, off = divmod(j, 4)
                if h % 2 == 0:
                    return qkv_t[(2, b * NQB + sb)][0:64,
                                                    off * 128:(off + 1) * 128]
                return kv2_t[b * NQB + sb][64:128, off * 128:(off + 1) * 128]

            vexts = {}

            # ---- phase 1 (QKV projection), one 512-row s-block at a time
            with (
                tc.tile_pool(name="wqp", bufs=1) as wqp,
                tc.tile_pool(name="xtp", bufs=2) as xtp,
            ):
                wq_t = wqp.tile([128, NKC, 384], F16)
                wq_r = wq.rearrange("(nk p) m -> p nk m", p=128)
                nc.sync.dma_start(wq_t[:, 0:1], wq_r[:, 0:1])
                nc.sync.dma_start(wq_t[:, 1:3], wq_r[:, 1:3])
                nc.sync.dma_start(wq_t[:, 3:8], wq_r[:, 3:8])
                nc.sync.dma_start(wq_t[:, 8:NKC], wq_r[:, 8:NKC])

                # phase-1 work for a block is split into single-op closures
                # so it can be drip-fed between attention pairs of the
                # PREVIOUS block (fills PE during Act-bound stretches and
                # keeps the p-state hot).
                pend_p1 = deque()

                def phase1_closures(sb):
                    xt = xtp.tile([128, NKC, 512], F16, tag="xt",
                                  name=f"xt{sb}")
                    sl = slice(sb * 512, (sb + 1) * 512)
                    if sb == 0:
                        # split across two idle DMA rings at startup
                        nc.gpsimd.dma_start(xt[:, 0:1], xT_r[:, 0:1, sl])
                        nc.scalar.dma_start(xt[:, 8:10], xT_r[:, 8:10, sl])
                        nc.gpsimd.dma_start(xt[:, 1:2], xT_r[:, 1:2, sl])
                        nc.gpsimd.dma_start(xt[:, 2:4], xT_r[:, 2:4, sl])
                        nc.scalar.dma_start(xt[:, 10:NKC], xT_r[:, 10:NKC, sl])
                        nc.gpsimd.dma_start(xt[:, 4:8], xT_r[:, 4:8, sl])
                    else:
                        nc.gpsimd.dma_start(xt[:], xT_r[:, :, sl])
                    ops = []
                    for m in (2, 0, 1):  # kv first: feeds vext early
                        cell = {}

                        def mk_mm(m, k, cell):
                            def mm():
                                if k == 0:
                                    cell["acc"] = ps_qkv.tile(
                                        [128, 512], F32, tag="qkv",
                                        name=f"qkvacc{sb}_{m}")
                                nc.tensor.matmul(
                                    cell["acc"][:],
                                    wq_t[:, k, m * 128:(m + 1) * 128],
                                    xt[:, k, :],
                                    start=(k == 0),
                                    stop=(k == NKC - 1),
                                )
                            return mm

                        for k in range(NKC):
                            ops.append(mk_mm(m, k, cell))

                        def mk_copy(m, cell):
                            def cp():
                                nc.vector.tensor_copy(
                                    qkv_t[(m, sb)][:], cell["acc"][:])
                                if m == 2:
                                    nc.sync.dma_start(
                                        kv2_t[sb][64:128, :],
                                        qkv_t[(2, sb)][0:64, :])
                            return cp

                        ops.append(mk_copy(m, cell))

                    b, sqb = divmod(sb, NQB)
                    for jo in range(4):
                        j = sqb * 4 + jo
                        vexts[(b, j)] = vextp.tile(
                            [128, 65], F16, tag="vext", name=f"vx_{b}_{j}")

                        def mk_vext(jo, j):
                            def vext():
                                vx = vexts[(b, j)]
                                tp = ps_st.tile([128, 64], F16, tag="st",
                                                name=f"vt_{b}_{j}")
                                nc.tensor.transpose(
                                    tp[:],
                                    qkv_t[(2, sb)][64:128,
                                                   jo * 128:(jo + 1) * 128],
                                    idn[64:128, 64:128],
                                )
                                nc.vector.tensor_copy(vx[:, 0:64], tp[:])
                                nc.vector.memset(vx[:, 64:65], 1.0)
                            return vext

                        ops.append(mk_vext(jo, j))
                    return ops

                # ---- attention machinery
                ost_tiles = {}
                ost_count = {}

                def get_ost(h, b):
                    if (h, b) not in ost_tiles:
                        ost_tiles[(h, b)] = ostp.tile(
                            [128, 16, HD], F16, tag="ost", name=f"ost{h}_{b}")
                        ost_count[(h, b)] = 0
                    return ost_tiles[(h, b)]

                def finish_ost(h, b, half=None):
                    # half=0: rows qt 0:8 eagerly; half=1: qt 8:16 (tile
                    # freed).  Splitting lets the collective start sooner
                    # after the last chain.
                    if half == 0:
                        ost = ost_tiles[(h, b)]
                    else:
                        ost = ost_tiles.pop((h, b))
                    buf = a2a_in[0] if b == 0 else a2a_in[(1, h // 2)]
                    co = 64 * h if b == 0 else 64 * (h % 2)
                    dst = buf.rearrange("(j q2 p) c -> p j q2 c",
                                        p=128, q2=2)
                    if half == 0:
                        nc.sync.dma_start(dst[:, 0:4, :, co:co + 64],
                                          ost[:, 0:8])
                    else:
                        nc.sync.dma_start(dst[:, 4:8, :, co:co + 64],
                                          ost[:, 8:16])

                def make_chain(h, b, sqb, qco, pts, offs):
                    qt = 4 * sqb + qco
                    jhi = qt if causal else NT - 1

                    def chain():
                        av = ps_av.tile([128, 65], F32, tag="av",
                                        name=f"av{h}_{b}_{qt}")
                        for j in range(jhi + 1):
                            c0 = qco * 128 - offs[j]
                            nc.tensor.matmul(
                                av[:],
                                pts[j][:, c0:c0 + 128],
                                vexts[(b, j)][:],
                                start=(j == 0),
                                stop=(j == jhi),
                                skip_group_check=True,
                            )
                        rsr = rsp.tile([128, 1], F32, tag="rsr",
                                       name=f"rsr{h}_{b}_{qt}")
                        nc.vector.reciprocal_approx_fast(rsr[:], av[:, 64:65])
                        ost = get_ost(h, b)
                        nc.vector.tensor_scalar_mul(
                            ost[:, qt, :], av[:, 0:64], rsr[:])
                        ost_count[(h, b)] += 1
                        if ost_count[(h, b)] == 8:
                            finish_ost(h, b, half=0)
                        elif ost_count[(h, b)] == 16:
                            finish_ost(h, b, half=1)

                    return chain

                def attn_block(h, b, sqb, tmod=2, last=False):
                    jmax = 4 * sqb + 3 if causal else NT - 1
                    ndiag0 = 4 * sqb if causal else jmax + 1
                    pts = {}
                    offs = {}
                    get_ost(h, b)
                    npair = (jmax + 1) // 2

                    def qk_pair(gi):
                        stp = ps_st.tile([128, 2, 512], F32, tag="st",
                                         name=f"st{h}_{b}_{sqb}_{gi}")
                        for idx in range(2):
                            j = 2 * gi + idx
                            off = (max(0, j * 128 - sqb * 512)
                                   if causal else 0)
                            n = 512 - off
                            nc.tensor.matmul(
                                stp[:, idx, :n],
                                k_ap(h, b, j),
                                q_ap(h, b, sqb, off, 512),
                                start=True,
                                stop=True,
                            )
                            offs[j] = off
                        return stp

                    def exp_pair(gi, stp):
                        j0 = 2 * gi
                        pt2 = ptp.tile([128, 2, 512], F16, tag="pt",
                                       name=f"pt{h}_{b}_{sqb}_{gi}")
                        if j0 + 1 < ndiag0:
                            # both halves unmasked: one fused activation
                            nc.scalar.activation(
                                pt2[:], stp[:], AF.Exp, scale=SCALE)
                        else:
                            for idx in range(2):
                                j = j0 + idx
                                n = 512 - offs[j]
                                nc.scalar.activation(
                                    pt2[:, idx, :n], stp[:, idx, :n],
                                    AF.Exp, scale=SCALE)
                                if causal and j >= ndiag0:
                                    nc.vector.tensor_tensor(
                                        pt2[:, idx, 0:128],
                                        pt2[:, idx, 0:128], ind_t[:], MUL)
                        pts[j0] = pt2[:, 0, :]
                        pts[j0 + 1] = pt2[:, 1, :]

                    stps = {0: qk_pair(0)}
                    for gi in range(npair):
                        if gi + 1 < npair:
                            stps[gi + 1] = qk_pair(gi + 1)
                        exp_pair(gi, stps.pop(gi))
                        for idx in range(2):
                            j = 2 * gi + idx
                            pump(pend_a, 4)
                            for _ in range(2):
                                if pend_p1:
                                    pend_p1.popleft()()
                            if (not pend_p1 and pend_tail
                                    and j % tmod == 0
                                    and not (last and gi >= npair - 3)):
                                pend_tail.popleft()()
                            if causal and j >= ndiag0:
                                pend_a.append(
                                    make_chain(h, b, sqb, j - ndiag0,
                                               pts, offs))
                    if not causal:
                        for qco in range(4):
                            pend_a.append(
                                make_chain(h, b, sqb, qco, pts, offs))
                        pump(pend_a, 4)

                def emit_a2a(key):
                    nc.gpsimd.collective_compute(
                        "AllToAll",
                        mybir.AluOpType.bypass,
                        replica_groups=[list(range(NCORE))],
                        ins=[a2a_in[key].opt()],
                        outs=[a2a_out[key].opt()],
                    )

                # ================= emission =================
                wot = {}

                def load_wo(eb, pool, queue):
                    w = pool.tile([128, NKC, 512], F16, tag="wo",
                                  name=f"wo{eb}")
                    queue.dma_start(
                        w[:], wo_r[:, :, eb * 512:(eb + 1) * 512])
                    wot[eb] = w

                ot_tiles = {}
                pend_tail = deque()

                def emit_ot(b):
                    ot = otp.tile([128, 2, 8, 256], F16, tag="ot",
                                  name=f"ot{b}")
                    src = a2a_out[b].rearrange("(j q2 p) c -> p q2 j c",
                                               p=128, q2=2)
                    nc.gpsimd.dma_start(ot[:, 0], src[:, 0])
                    nc.gpsimd.dma_start(ot[:, 1], src[:, 1])
                    aoT = aotp.tile([128, 2, 16, 128], F16, tag="aot",
                                    name=f"aoT{b}")
                    ot_tiles[b] = (ot, aoT)

                def emit_oth(pr):
                    # batch-1 pair buffer: chans [0:128] of kt = 2j + pr
                    ot = otp.tile([128, 2, 8, 128], F16, tag="oth",
                                  name=f"ot1p{pr}")
                    src = a2a_out[(1, pr)].rearrange(
                        "(j q2 p) c -> p q2 j c", p=128, q2=2)
                    nc.gpsimd.dma_start(ot[:, 0], src[:, 0])
                    nc.scalar.dma_start(ot[:, 1], src[:, 1])
                    if 1 not in ot_tiles:
                        # [p, ss, pair, j, c]: kt = 2j + pair
                        aoT = aotp.tile([128, 2, 2, 8, 128], F16, tag="aot",
                                        name="aoT1")
                        ot_tiles[1] = (None, aoT)
                    return ot

                def tp4(b, ss, gg):
                    # 4 PE transposes into one psum tile + one DVE copy
                    def f():
                        ot, aoT = ot_tiles[b]
                        tpt = ps_st.tile([128, 4, 128], F16, tag="st",
                                         name=f"tp{b}_{ss}_{gg}")
                        for i in range(4):
                            kt = gg * 4 + i
                            j, ch = divmod(kt, 2)
                            nc.tensor.transpose(
                                tpt[:, i, :],
                                ot[:, ss, j, ch * 128:(ch + 1) * 128],
                                idn[:],
                            )
                        nc.vector.tensor_copy(
                            aoT[:, ss, gg * 4:gg * 4 + 4, :], tpt[:])
                    return f

                def tp4h(oth, pr, ss, jg):
                    # 4 transposes of b1 pair pr, source j in [4jg, 4jg+4)
                    def f():
                        _, aoT = ot_tiles[1]
                        tpt = ps_st.tile([128, 4, 128], F16, tag="st",
                                         name=f"tph{pr}_{ss}_{jg}")
                        for i in range(4):
                            j = jg * 4 + i
                            nc.tensor.transpose(
                                tpt[:, i, :], oth[:, ss, j, :], idn[:])
                        nc.vector.tensor_copy(
                            aoT[:, ss, pr, jg * 4:jg * 4 + 4, :], tpt[:])
                    return f

                parts = {}

                def half_chunk(eb, ss, pr):
                    # b1 out-proj over kt = pr, pr+2, ..., pr+14.  Pair 0
                    # parks the partial in fp16 SBUF; pair 1 adds it back.
                    def f():
                        _, aoT = ot_tiles[1]
                        ops = ps_qkv.tile([128, 512], F32, tag="qkv",
                                          name=f"opsh{eb}_{ss}_{pr}")
                        for jj in range(8):
                            nc.tensor.matmul(
                                ops[:],
                                aoT[:, ss, pr, jj, :],
                                wot[eb][:, 2 * jj + pr, :],
                                start=(jj == 0),
                                stop=(jj == 7),
                            )
                        if pr == 0:
                            part = partp.tile([128, 512], F16, tag="part",
                                              name=f"part{eb}_{ss}")
                            nc.vector.tensor_copy(part[:], ops[:])
                            parts[(eb, ss)] = part
                        else:
                            og = outstp.tile([128, 512], F32, tag="out",
                                             name=f"ogh{eb}_{ss}")
                            nc.vector.tensor_tensor(
                                og[:], parts[(eb, ss)][:], ops[:], ADD)
                            nc.sync.dma_start(
                                out[256 + ss * 128:256 + ss * 128 + 128,
                                    eb * 512:(eb + 1) * 512],
                                og[:],
                            )
                    return f

                def chunk_quarters(b, eb, ss):
                    # out-proj chunk split into 4 pumpable quarters
                    cell = {}

                    def mk_q(q):
                        def f():
                            _, aoT = ot_tiles[b]
                            if q == 0:
                                cell["ops"] = ps_qkv.tile(
                                    [128, 512], F32, tag="qkv",
                                    name=f"ops{b}_{eb}_{ss}")
                            for k in range(4 * q, 4 * q + 4):
                                nc.tensor.matmul(
                                    cell["ops"][:],
                                    aoT[:, ss, k, :],
                                    wot[eb][:, k, :],
                                    start=(k == 0),
                                    stop=(k == NKC - 1),
                                )
                            if q == 3:
                                og = outstp.tile([128, 512], F32, tag="out",
                                                 name=f"og{b}_{eb}_{ss}")
                                nc.vector.tensor_copy(og[:], cell["ops"][:])
                                nc.sync.dma_start(
                                    out[b * 256 + ss * 128:
                                        b * 256 + ss * 128 + 128,
                                        eb * 512:(eb + 1) * 512],
                                    og[:],
                                )
                        return f

                    return [mk_q(q) for q in range(4)]

                for sb in range(NSB):
                    if sb == 0:
                        for op in phase1_closures(0):
                            op()
                    pump(pend_p1)  # leftover phase-1 ops of this block
                    if sb + 1 < NSB:
                        pend_p1.extend(phase1_closures(sb + 1))
                    b, sqb = divmod(sb, NQB)
                    for h in (range(4) if b == 0 else (0, 1, 2)):
                        attn_block(h, b, sqb)
                        if sb == NSB - 1 and h == 1:
                            # heads 0,1 of batch 1 complete -> pair-0
                            # collective overlaps h2/h3 tail attention
                            pump(pend_a)
                            emit_a2a((1, 0))
                    if sb == NQB - 1:
                        # batch 0 attention fully emitted; chains drain in
                        # the pump lag -> collective as soon as osts land
                        pump(pend_a)
                        emit_a2a(0)
                        emit_ot(0)
                        load_wo(0, wopA, nc.sync)
                        load_wo(1, wopA, nc.sync)
                        # batch-0 epilogue work for the preloaded k-halves
                        # becomes PE filler during batch-1 attention
                        for ss in range(2):
                            for gg in range(4):
                                pend_tail.append(tp4(0, ss, gg))
                            for eb in (0, 1):
                                pend_tail.extend(chunk_quarters(0, eb, ss))
            # ---- batch-1 head 3 + epilogue.  Pair-0's collective runs
            # during this attention; its out-proj half covers pair-1's.
            # Unpumped b0 quarters become the reserve for the final window.
            with tc.tile_pool(name="wopB", bufs=2) as wopB:
                load_wo(2, wopB, nc.sync)
                load_wo(3, wopB, nc.sync)
                for ss in range(2):
                    for eb in (2, 3):
                        pend_tail.extend(chunk_quarters(0, eb, ss))
                # throttled pumping: keep most of the b0 reserve for the
                # pair-1 collective window; none in the last block so the
                # final chains' normalize + ost DMA aren't queued behind
                # tail copies
                for sqb in range(NQB):
                    attn_block(3, 1, sqb,
                               tmod=3 if sqb < NQB - 1 else 99)
                pump(pend_a)
                emit_a2a((1, 1))

                # pair-0 epilogue half (fills pair-1's collective window),
                # then the held-back b0 chunks cover the ot1p1 latency
                ot_p0 = emit_oth(0)
                for ss in range(2):
                    for jg in range(2):
                        tp4h(ot_p0, 0, ss, jg)()
                for ss in range(2):
                    for eb in range(4):
                        half_chunk(eb, ss, 0)()
                pump(pend_tail)
                # pair-1 epilogue half
                ot_p1 = emit_oth(1)
                for ss in range(2):
                    for jg in range(2):
                        tp4h(ot_p1, 1, ss, jg)()
                for ss in range(2):
                    for eb in range(4):
                        half_chunk(eb, ss, 1)()

    nc.compile()
    return nc


def _program(causal: bool):
    if causal not in _CACHE:
        _CACHE[causal] = _build(causal)
    return _CACHE[causal]


def _prep_in_maps(x, W_qkv, W_out):
    xT = x.reshape(S_TOT, DIM).T.astype(np.float16)  # [c, s] contiguous
    ind_np = (np.arange(128)[None, :] >= np.arange(128)[:, None]).astype(np.float16)

    # W_out cols permuted to the received-block channel order: block j
    # (256 chans) carries core j's heads {j, j+8, j+16, j+24}.
    wt = W_out.T.reshape(N_HEADS, HD, DIM)
    woT = np.ascontiguousarray(np.concatenate(
        [wt[[j, j + 8, j + 16, j + 24]].reshape(4 * HD, DIM)
         for j in range(NCORE)]
    )).astype(np.float16)

    in_maps = []
    for c in range(NCORE):
        rows = np.concatenate(
            [np.arange(h * HD, (h + 1) * HD) for h in (c, c + 8, c + 16, c + 24)]
            + [np.arange(2048 + c * HD, 2048 + (c + 1) * HD),
               np.arange(2560 + c * HD, 2560 + (c + 1) * HD)]
        )
        wq_shard = np.ascontiguousarray(W_qkv[rows].T).astype(np.float16)
        in_maps.append({"xT": xT, "wq": wq_shard, "wo": woT, "ind": ind_np})
    return in_maps


def kernel(**inputs) -> np.ndarray:
    from concourse.bass_utils import run_bass_kernel_spmd

    x = np.asarray(inputs["x"], np.float32)
    W_qkv = np.asarray(inputs["W_qkv"], np.float32)
    W_out = np.asarray(inputs["W_out"], np.float32)
    mask = np.asarray(inputs["mask"], np.float32)

    m2 = mask.reshape(SEQ, SEQ)
    iu = np.triu_indices(SEQ, 1)
    causal = bool((m2[iu] <= -1e8).all()) and bool((np.tril(m2) == 0.0).all())
    dense = bool((m2 == 0.0).all())
    if not (causal or dense):
        raise NotImplementedError("only causal or all-zero masks supported")

    nc = _program(causal)
    in_maps = _prep_in_maps(x, W_qkv, W_out)
    res = run_bass_kernel_spmd(nc, in_maps, list(range(NCORE))).results
    # core c owns rows [256c, 256c+256) of each batch
    full = np.empty((S_TOT, DIM), np.float32)
    for c in range(NCORE):
        o = res[c]["out"]
        full[256 * c:256 * c + 256] = o[0:256]
        full[SEQ + 256 * c:SEQ + 256 * c + 256] = o[256:512]
    return np.ascontiguousarray(full.reshape(BATCH, SEQ, DIM), dtype=np.float32)


# revision 46
# speedup vs baseline: 1.0054x; 1.0053x over previous
"""Trainium2 Bass kernel for nn_AcceleratedAttention (GQA transformer block:
fused QKV proj -> causal GQA attention -> out proj), tensor-parallel over
heads on 8 NeuronCores.

Sharding (per core c of 8):
  - q heads {c, c+8, c+16, c+24} and kv head c (all four q heads share kv
    head c, so K/V are computed once per core).
  - Per 512-row s-block: QKV projection, then attention for ALL FOUR heads
    interleaved (QK pairs -> exp on Act -> AV chains pumped with lag), so
    the Activation engine load is spread uniformly and PE stays the
    bottleneck everywhere.
  - Attention runs with scores transposed ([kv, q] tiles).  The AV product
    uses the probability tile as the STATIONARY operand:
    out[q, hd] = pt[kv, q].T @ [V | 1], a 65-column moving matmul at full
    128-row contraction; the ones column delivers the softmax row-sum in
    psum column 64 (normalization = per-partition reciprocal+scale on DVE).
  - ONE merged AllToAll per batch ([2048 rows, 256 chans]; 256-row chunks)
    — the collective cost model charges a flat ~15us per collective, so
    fewer+earlier collectives win.  Core c ends up owning rows
    [256c, 256c+256) of each batch.
  - Epilogue per batch: received [row, chan] blocks are PE-transposed to
    [chan, row]; out-proj = 8 chunks of 16 accumulating matmuls (full
    2048-contraction in one psum bank, no fp16 partial accumulate).
    W_out tiles are preloaded during the last attention blocks.

All matmuls run in fp16 (1 cycle/row on PE; psum fp32).
"""
import sys
from collections import deque

sys.path.insert(0, "/opt/trn_rl_repo")

import numpy as np

DIM = 2048
N_HEADS = 32
N_KV = 8
HD = 64
BATCH = 2
SEQ = 2048
NCORE = 8
S_TOT = BATCH * SEQ          # 4096
S_SLICE = S_TOT // NCORE     # 512
NKC = DIM // 128             # 16 contraction tiles
NSB = S_TOT // 512           # 8 s-blocks for qkv proj
NT = SEQ // 128              # 16 kv tiles per batch
NQB = SEQ // 512             # 4 sq blocks per batch
SCALE = 0.125                # 1/sqrt(64)

_CACHE = {}
TRACE_SIM = False


def _build(causal: bool):
    import concourse.mybir as mybir
    import concourse.tile as tile
    from concourse import bacc
    from concourse.masks import make_identity

    F32 = mybir.dt.float32
    F16 = mybir.dt.float16
    AF = mybir.ActivationFunctionType
    MUL = mybir.AluOpType.mult
    ADD = mybir.AluOpType.add

    nc = bacc.Bacc()
    xT = nc.declare_dram_parameter("xT", [DIM, S_TOT], F16, isOutput=False)
    wq = nc.declare_dram_parameter("wq", [DIM, 384], F16, isOutput=False)
    wo = nc.declare_dram_parameter("wo", [DIM, DIM], F16, isOutput=False)
    ind = nc.declare_dram_parameter("ind", [128, 128], F16, isOutput=False)
    # rows [0,256) = batch0 rows [256c,256c+256), rows [256,512) = batch1
    out = nc.declare_dram_parameter("out", [S_SLICE, DIM], F32, isOutput=True)

    with tile.TileContext(nc, trace_sim=TRACE_SIM) as tc:
        with (
            tc.tile_pool(name="consts", bufs=1) as consts,
            tc.tile_pool(name="qkvp", bufs=1) as qkvp,
            tc.tile_pool(name="vextp", bufs=32) as vextp,
            tc.tile_pool(name="ptp", bufs=17) as ptp,
            tc.tile_pool(name="rsp", bufs=4) as rsp,
            tc.tile_pool(name="ostp", bufs=5) as ostp,
            tc.tile_pool(name="otp", bufs=1) as otp,
            tc.tile_pool(name="aotp", bufs=2) as aotp,
            tc.tile_pool(name="outstp", bufs=3) as outstp,
            tc.tile_pool(name="partp", bufs=8) as partp,
            tc.tile_pool(name="wopA", bufs=2) as wopA,
            tc.tile_pool(name="ps_qkv", bufs=2, space="PSUM") as ps_qkv,
            tc.tile_pool(name="ps_st", bufs=2, space="PSUM") as ps_st,
            tc.tile_pool(name="ps_av", bufs=2, space="PSUM") as ps_av,
            tc.tile_pool(name="dram", bufs=1, space="DRAM") as dram,
        ):
            # ---- constants
            idn = consts.tile([128, 128], F16)
            make_identity(nc, idn[:])
            ind_t = consts.tile([128, 128], F16)
            nc.sync.dma_start(ind_t[:], ind[:])

            # Merged AllToAll for batch 0 ([2048 rows, 256 chans]); batch 1
            # split by head pair into two [2048, 128] collectives so the
            # first pair's out-proj half covers the last collective's
            # window.  256-row chunks -> core c gets rows [256c, 256c+256).
            a2a_in = {}
            a2a_out = {}
            a2a_in[0] = dram.tile([SEQ, 256], F16, name="a2ain0")
            a2a_out[0] = dram.tile([SEQ, 256], F16, name="a2aout0")
            for pr in range(2):
                a2a_in[(1, pr)] = dram.tile([SEQ, 128], F16,
                                            name=f"a2ain1p{pr}")
                a2a_out[(1, pr)] = dram.tile([SEQ, 128], F16,
                                             name=f"a2aout1p{pr}")

            xT_r = xT.rearrange("(nk p) s -> p nk s", p=128)
            wo_r = wo.rearrange("(nk p) e -> p nk e", p=128)

            # deferred AV chains, pumped with ~4-item lag behind exp
            pend_a = deque()

            def pump(q, drain_to=0):
                while len(q) > drain_to:
                    q.popleft()()

            # ---- qkv tiles
            qkv_t = {(m, sb): qkvp.tile([128, 512], F16, tag=f"qkv{m}_{sb}",
                                        name=f"qkv{m}_{sb}")
                     for m in range(3) for sb in range(NSB)}
            # K replicated at partitions 64:128 so odd heads' Q (upper
            # partition half) sees a matching base partition.
            kv2_t = {sb: qkvp.tile([128, 512], F16, tag=f"kv2_{sb}",
                                   name=f"kv2_{sb}")
                     for sb in range(NSB)}

            def q_ap(h, b, sqb, c0, c1):
                t = qkv_t[(h // 2, b * NQB + sqb)]
                base = 64 * (h % 2)
                return t[base:base + 64, c0:c1]

            def k_ap(h, b, j):
                sb, off = divmod(j, 4)
                if h % 2 == 0:
                    return qkv_t[(2, b * NQB + sb)][0:64,
                                                    off * 128:(off + 1) * 128]
                return kv2_t[b * NQB + sb][64:128, off * 128:(off + 1) * 128]

            vexts = {}

            # ---- phase 1 (QKV projection), one 512-row s-block at a time
            with (
                tc.tile_pool(name="wqp", bufs=1) as wqp,
                tc.tile_pool(name="xtp", bufs=2) as xtp,
            ):
                wq_t = wqp.tile([128, NKC, 384], F16)
                wq_r = wq.rearrange("(nk p) m -> p nk m", p=128)
                nc.sync.dma_start(wq_t[:, 0:1], wq_r[:, 0:1])
                nc.sync.dma_start(wq_t[:, 1:3], wq_r[:, 1:3])
                nc.sync.dma_start(wq_t[:, 3:8], wq_r[:, 3:8])
                nc.sync.dma_start(wq_t[:, 8:NKC], wq_r[:, 8:NKC])

                # phase-1 work for a block is split into single-op closures
                # so it can be drip-fed between attention pairs of the
                # PREVIOUS block (fills PE during Act-bound stretches and
                # keeps the p-state hot).
                pend_p1 = deque()

                def phase1_closures(sb):
                    xt = xtp.tile([128, NKC, 512], F16, tag="xt",
                                  name=f"xt{sb}")
                    sl = slice(sb * 512, (sb + 1) * 512)
                    if sb == 0:
                        # split across two idle DMA rings at startup
                        nc.gpsimd.dma_start(xt[:, 0:1], xT_r[:, 0:1, sl])
                        nc.scalar.dma_start(xt[:, 8:10], xT_r[:, 8:10, sl])
                        nc.gpsimd.dma_start(xt[:, 1:2], xT_r[:, 1:2, sl])
                        nc.gpsimd.dma_start(xt[:, 2:4], xT_r[:, 2:4, sl])
                        nc.scalar.dma_start(xt[:, 10:NKC], xT_r[:, 10:NKC, sl])
                        nc.gpsimd.dma_start(xt[:, 4:8], xT_r[:, 4:8, sl])
                    else:
                        nc.gpsimd.dma_start(xt[:], xT_r[:, :, sl])
                    ops = []
                    for m in (2, 0, 1):  # kv first: feeds vext early
                        cell = {}

                        def mk_mm(m, k, cell):
                            def mm():
                                if k == 0:
                                    cell["acc"] = ps_qkv.tile(
                                        [128, 512], F32, tag="qkv",
                                        name=f"qkvacc{sb}_{m}")
                                nc.tensor.matmul(
                                    cell["acc"][:],
                                    wq_t[:, k, m * 128:(m + 1) * 128],
                                    xt[:, k, :],
                                    start=(k == 0),
                                    stop=(k == NKC - 1),
                                )
                            return mm

                        for k in range(NKC):
                            ops.append(mk_mm(m, k, cell))

                        def mk_copy(m, cell):
                            def cp():
                                nc.vector.tensor_copy(
                                    qkv_t[(m, sb)][:], cell["acc"][:])
                                if m == 2:
                                    nc.sync.dma_start(
                                        kv2_t[sb][64:128, :],
                                        qkv_t[(2, sb)][0:64, :])
                            return cp

                        ops.append(mk_copy(m, cell))

                    b, sqb = divmod(sb, NQB)
                    for jo in range(4):
                        j = sqb * 4 + jo
                        vexts[(b, j)] = vextp.tile(
                            [128, 65], F16, tag="vext", name=f"vx_{b}_{j}")

                        def mk_vext(jo, j):
                            def vext():
                                vx = vexts[(b, j)]
                                tp = ps_st.tile([128, 64], F16, tag="st",
                                                name=f"vt_{b}_{j}")
                                nc.tensor.transpose(
                                    tp[:],
                                    qkv_t[(2, sb)][64:128,
                                                   jo * 128:(jo + 1) * 128],
                                    idn[64:128, 64:128],
                                )
                                nc.vector.tensor_copy(vx[:, 0:64], tp[:])
                                nc.vector.memset(vx[:, 64:65], 1.0)
                            return vext

                        ops.append(mk_vext(jo, j))
                    return ops

                # ---- attention machinery
                ost_tiles = {}
                ost_count = {}

                def get_ost(h, b):
                    if (h, b) not in ost_tiles:
                        ost_tiles[(h, b)] = ostp.tile(
                            [128, 16, HD], F16, tag="ost", name=f"ost{h}_{b}")
                        ost_count[(h, b)] = 0
                    return ost_tiles[(h, b)]

                def finish_ost(h, b, half=None):
                    # half=0: rows qt 0:8 eagerly; half=1: qt 8:16 (tile
                    # freed).  Splitting lets the collective start sooner
                    # after the last chain.
                    if half == 0:
                        ost = ost_tiles[(h, b)]
                    else:
                        ost = ost_tiles.pop((h, b))
                    buf = a2a_in[0] if b == 0 else a2a_in[(1, h // 2)]
                    co = 64 * h if b == 0 else 64 * (h % 2)
                    dst = buf.rearrange("(j q2 p) c -> p j q2 c",
                                        p=128, q2=2)
                    if half == 0:
                        nc.sync.dma_start(dst[:, 0:4, :, co:co + 64],
                                          ost[:, 0:8])
                    else:
                        nc.sync.dma_start(dst[:, 4:8, :, co:co + 64],
                                          ost[:, 8:16])

                def make_chain(h, b, sqb, qco, pts, offs):
                    qt = 4 * sqb + qco
                    jhi = qt if causal else NT - 1

                    def chain():
                        av = ps_av.tile([128, 65], F32, tag="av",
                                        name=f"av{h}_{b}_{qt}")
                        for j in range(jhi + 1):
                            c0 = qco * 128 - offs[j]
                            nc.tensor.matmul(
                                av[:],
                                pts[j][:, c0:c0 + 128],
                                vexts[(b, j)][:],
                                start=(j == 0),
                                stop=(j == jhi),
                                skip_group_check=True,
                            )
                        rsr = rsp.tile([128, 1], F32, tag="rsr",
                                       name=f"rsr{h}_{b}_{qt}")
                        nc.vector.reciprocal_approx_fast(rsr[:], av[:, 64:65])
                        ost = get_ost(h, b)
                        nc.vector.tensor_scalar_mul(
                            ost[:, qt, :], av[:, 0:64], rsr[:])
                        ost_count[(h, b)] += 1
                        if ost_count[(h, b)] == 8:
                            finish_ost(h, b, half=0)
                        elif ost_count[(h, b)] == 16:
                            finish_ost(h, b, half=1)

                    return chain

                def attn_block(h, b, sqb, tmod=2, last=False):
                    jmax = 4 * sqb + 3 if causal else NT - 1
                    ndiag0 = 4 * sqb if causal else jmax + 1
                    pts = {}
                    offs = {}
                    get_ost(h, b)
                    npair = (jmax + 1) // 2

                    def qk_pair(gi):
                        stp = ps_st.tile([128, 2, 512], F32, tag="st",
                                         name=f"st{h}_{b}_{sqb}_{gi}")
                        for idx in range(2):
                            j = 2 * gi + idx
                            off = (max(0, j * 128 - sqb * 512)
                                   if causal else 0)
                            n = 512 - off
                            nc.tensor.matmul(
                                stp[:, idx, :n],
                                k_ap(h, b, j),
                                q_ap(h, b, sqb, off, 512),
                                start=True,
                                stop=True,
                            )
                            offs[j] = off
                        return stp

                    def exp_pair(gi, stp):
                        j0 = 2 * gi
                        pt2 = ptp.tile([128, 2, 512], F16, tag="pt",
                                       name=f"pt{h}_{b}_{sqb}_{gi}")
                        if j0 + 1 < ndiag0:
                            # both halves unmasked: one fused activation
                            nc.scalar.activation(
                                pt2[:], stp[:], AF.Exp, scale=SCALE)
                        else:
                            for idx in range(2):
                                j = j0 + idx
                                n = 512 - offs[j]
                                nc.scalar.activation(
                                    pt2[:, idx, :n], stp[:, idx, :n],
                                    AF.Exp, scale=SCALE)
                                if causal and j >= ndiag0:
                                    nc.vector.tensor_tensor(
                                        pt2[:, idx, 0:128],
                                        pt2[:, idx, 0:128], ind_t[:], MUL)
                        pts[j0] = pt2[:, 0, :]
                        pts[j0 + 1] = pt2[:, 1, :]

                    stps = {0: qk_pair(0)}
                    for gi in range(npair):
                        if gi + 1 < npair:
                            stps[gi + 1] = qk_pair(gi + 1)
                        exp_pair(gi, stps.pop(gi))
                        for idx in range(2):
                            j = 2 * gi + idx
                            pump(pend_a, 4)
                            for _ in range(2):
                                if pend_p1:
                                    pend_p1.popleft()()
                            if (not pend_p1 and pend_tail
                                    and j % tmod == 0
                                    and not (last and gi >= npair - 3)):
                                pend_tail.popleft()()
                            if causal and j >= ndiag0:
                                pend_a.append(
                                    make_chain(h, b, sqb, j - ndiag0,
                                               pts, offs))
                    if not causal:
                        for qco in range(4):
                            pend_a.append(
                                make_chain(h, b, sqb, qco, pts, offs))
                        pump(pend_a, 4)

                def emit_a2a(key):
                    nc.gpsimd.collective_compute(
                        "AllToAll",
                        mybir.AluOpType.bypass,
                        replica_groups=[list(range(NCORE))],
                        ins=[a2a_in[key].opt()],
                        outs=[a2a_out[key].opt()],
                    )

                # ================= emission =================
                wot = {}

                def load_wo(eb, pool, queue):
                    w = pool.tile([128, NKC, 512], F16, tag="wo",
                                  name=f"wo{eb}")
                    queue.dma_start(
                        w[:], wo_r[:, :, eb * 512:(eb + 1) * 512])
                    wot[eb] = w

                ot_tiles = {}
                pend_tail = deque()

                def emit_ot(b):
                    ot = otp.tile([128, 2, 8, 256], F16, tag="ot",
                                  name=f"ot{b}")
                    src = a2a_out[b].rearrange("(j q2 p) c -> p q2 j c",
                                               p=128, q2=2)
                    nc.gpsimd.dma_start(ot[:, 0], src[:, 0])
                    nc.gpsimd.dma_start(ot[:, 1], src[:, 1])
                    aoT = aotp.tile([128, 2, 16, 128], F16, tag="aot",
                                    name=f"aoT{b}")
                    ot_tiles[b] = (ot, aoT)

                def emit_oth(pr):
                    # batch-1 pair buffer: chans [0:128] of kt = 2j + pr
                    ot = otp.tile([128, 2, 8, 128], F16, tag="oth",
                                  name=f"ot1p{pr}")
                    src = a2a_out[(1, pr)].rearrange(
                        "(j q2 p) c -> p q2 j c", p=128, q2=2)
                    nc.gpsimd.dma_start(ot[:, 0], src[:, 0])
                    nc.scalar.dma_start(ot[:, 1], src[:, 1])
                    if 1 not in ot_tiles:
                        # [p, ss, pair, j, c]: kt = 2j + pair
                        aoT = aotp.tile([128, 2, 2, 8, 128], F16, tag="aot",
                                        name="aoT1")
                        ot_tiles[1] = (None, aoT)
                    return ot

                def tp4(b, ss, gg):
                    # 4 PE transposes into one psum tile + one DVE copy
                    def f():
                        ot, aoT = ot_tiles[b]
                        tpt = ps_st.tile([128, 4, 128], F16, tag="st",
                                         name=f"tp{b}_{ss}_{gg}")
                        for i in range(4):
                            kt = gg * 4 + i
                            j, ch = divmod(kt, 2)
                            nc.tensor.transpose(
                                tpt[:, i, :],
                                ot[:, ss, j, ch * 128:(ch + 1) * 128],
                                idn[:],
                            )
                        nc.vector.tensor_copy(
                            aoT[:, ss, gg * 4:gg * 4 + 4, :], tpt[:])
                    return f

                def tp4h(oth, pr, ss, jg):
                    # 4 transposes of b1 pair pr, source j in [4jg, 4jg+4)
                    def f():
                        _, aoT = ot_tiles[1]
                        tpt = ps_st.tile([128, 4, 128], F16, tag="st",
                                         name=f"tph{pr}_{ss}_{jg}")
                        for i in range(4):
                            j = jg * 4 + i
                            nc.tensor.transpose(
                                tpt[:, i, :], oth[:, ss, j, :], idn[:])
                        nc.vector.tensor_copy(
                            aoT[:, ss, pr, jg * 4:jg * 4 + 4, :], tpt[:])
                    return f

                parts = {}

                def half_chunk(eb, ss, pr):
                    # b1 out-proj over kt = pr, pr+2, ..., pr+14.  Pair 0
                    # parks the partial in fp16 SBUF; pair 1 adds it back.
                    def f():
                        _, aoT = ot_tiles[1]
                        ops = ps_qkv.tile([128, 512], F32, tag="qkv",
                                          name=f"opsh{eb}_{ss}_{pr}")
                        for jj in range(8):
                            nc.tensor.matmul(
                                ops[:],
                                aoT[:, ss, pr, jj, :],
                                wot[eb][:, 2 * jj + pr, :],
                                start=(jj == 0),
                                stop=(jj == 7),
                            )
                        if pr == 0:
                            part = partp.tile([128, 512], F16, tag="part",
                                              name=f"part{eb}_{ss}")
                            nc.vector.tensor_copy(part[:], ops[:])
                            parts[(eb, ss)] = part
                        else:
                            og = outstp.tile([128, 512], F32, tag="out",
                                             name=f"ogh{eb}_{ss}")
                            nc.vector.tensor_tensor(
                                og[:], parts[(eb, ss)][:], ops[:], ADD)
                            nc.sync.dma_start(
                                out[256 + ss * 128:256 + ss * 128 + 128,
                                    eb * 512:(eb + 1) * 512],
                                og[:],
                            )
                    return f

                def chunk_quarters(b, eb, ss):
                    # out-proj chunk split into 4 pumpable quarters
                    cell = {}

                    def mk_q(q):
                        def f():
                            _, aoT = ot_tiles[b]
                            if q == 0:
                                cell["ops"] = ps_qkv.tile(
                                    [128, 512], F32, tag="qkv",
                                    name=f"ops{b}_{eb}_{ss}")
                            for k in range(4 * q, 4 * q + 4):
                                nc.tensor.matmul(
                                    cell["ops"][:],
                                    aoT[:, ss, k, :],
                                    wot[eb][:, k, :],
                                    start=(k == 0),
                                    stop=(k == NKC - 1),
                                )
                            if q == 3:
                                og = outstp.tile([128, 512], F32, tag="out",
                                                 name=f"og{b}_{eb}_{ss}")
                                nc.vector.tensor_copy(og[:], cell["ops"][:])
                                nc.sync.dma_start(
                                    out[b * 256 + ss * 128:
                                        b * 256 + ss * 128 + 128,
                                        eb * 512:(eb + 1) * 512],
                                    og[:],
                                )
                        return f

                    return [mk_q(q) for q in range(4)]

                for sb in range(NSB):
                    if sb == 0:
                        for op in phase1_closures(0):
                            op()
                    pump(pend_p1)  # leftover phase-1 ops of this block
                    if sb + 1 < NSB:
                        pend_p1.extend(phase1_closures(sb + 1))
                    b, sqb = divmod(sb, NQB)
                    for h in (range(4) if b == 0 else (0, 1, 2)):
                        attn_block(h, b, sqb)
                        if sb == NSB - 1 and h == 1:
                            # heads 0,1 of batch 1 complete -> pair-0
                            # collective overlaps h2/h3 tail attention
                            pump(pend_a)
                            emit_a2a((1, 0))
                    if sb == NQB - 1:
                        # batch 0 attention fully emitted; chains drain in
                        # the pump lag -> collective as soon as osts land
                        pump(pend_a)
                        emit_a2a(0)
                        emit_ot(0)
                        load_wo(0, wopA, nc.sync)
                        load_wo(1, wopA, nc.sync)
                        # batch-0 epilogue work for the preloaded k-halves
                        # becomes PE filler during batch-1 attention
                        for ss in range(2):
                            for gg in range(4):
                                pend_tail.append(tp4(0, ss, gg))
                            for eb in (0, 1):
                                pend_tail.extend(chunk_quarters(0, eb, ss))
            # ---- batch-1 head 3 + epilogue.  Pair-0's collective runs
            # during this attention; its out-proj half covers pair-1's.
            # Unpumped b0 quarters become the reserve for the final window.
            with tc.tile_pool(name="wopB", bufs=2) as wopB:
                load_wo(2, wopB, nc.sync)
                load_wo(3, wopB, nc.sync)
                for ss in range(2):
                    for eb in (2, 3):
                        pend_tail.extend(chunk_quarters(0, eb, ss))
                # throttled pumping: keep most of the b0 reserve for the
                # pair-1 collective window; none in the last block so the
                # final chains' normalize + ost DMA aren't queued behind
                # tail copies
                for sqb in range(NQB):
                    attn_block(3, 1, sqb,
                               tmod=3 if sqb < NQB - 1 else 99)
                pump(pend_a)
                emit_a2a((1, 1))

                # pair-0 epilogue half (fills pair-1's collective window),
                # then the held-back b0 chunks cover the ot1p1 latency
                ot_p0 = emit_oth(0)
                for ss in range(2):
                    for jg in range(2):
                        tp4h(ot_p0, 0, ss, jg)()
                for ss in range(2):
                    for eb in range(4):
                        half_chunk(eb, ss, 0)()
                pump(pend_tail)
                # PE warm-up filler: keep the tensor engine busy (and the
                # p-state at full clock) across the pair-1 collective's
                # landing latency; results are never read
                warm = ps_qkv.tile([128, 128], F16, tag="qkv", name="warm")
                for _ in range(105):
                    nc.tensor.transpose(warm[:], idn[:], idn[:])
                # pair-1 epilogue half
                ot_p1 = emit_oth(1)
                for ss in range(2):
                    for jg in range(2):
                        tp4h(ot_p1, 1, ss, jg)()
                for ss in range(2):
                    for eb in range(4):
                        half_chunk(eb, ss, 1)()

    nc.compile()
    return nc


def _program(causal: bool):
    if causal not in _CACHE:
        _CACHE[causal] = _build(causal)
    return _CACHE[causal]


def _prep_in_maps(x, W_qkv, W_out):
    xT = x.reshape(S_TOT, DIM).T.astype(np.float16)  # [c, s] contiguous
    ind_np = (np.arange(128)[None, :] >= np.arange(128)[:, None]).astype(np.float16)

    # W_out cols permuted to the received-block channel order: block j
    # (256 chans) carries core j's heads {j, j+8, j+16, j+24}.
    wt = W_out.T.reshape(N_HEADS, HD, DIM)
    woT = np.ascontiguousarray(np.concatenate(
        [wt[[j, j + 8, j + 16, j + 24]].reshape(4 * HD, DIM)
         for j in range(NCORE)]
    )).astype(np.float16)

    in_maps = []
    for c in range(NCORE):
        rows = np.concatenate(
            [np.arange(h * HD, (h + 1) * HD) for h in (c, c + 8, c + 16, c + 24)]
            + [np.arange(2048 + c * HD, 2048 + (c + 1) * HD),
               np.arange(2560 + c * HD, 2560 + (c + 1) * HD)]
        )
        wq_shard = np.ascontiguousarray(W_qkv[rows].T).astype(np.float16)
        in_maps.append({"xT": xT, "wq": wq_shard, "wo": woT, "ind": ind_np})
    return in_maps


def kernel(**inputs) -> np.ndarray:
    from concourse.bass_utils import run_bass_kernel_spmd

    x = np.asarray(inputs["x"], np.float32)
    W_qkv = np.asarray(inputs["W_qkv"], np.float32)
    W_out = np.asarray(inputs["W_out"], np.float32)
    mask = np.asarray(inputs["mask"], np.float32)

    m2 = mask.reshape(SEQ, SEQ)
    iu = np.triu_indices(SEQ, 1)
    causal = bool((m2[iu] <= -1e8).all()) and bool((np.tril(m2) == 0.0).all())
    dense = bool((m2 == 0.0).all())
    if not (causal or dense):
        raise NotImplementedError("only causal or all-zero masks supported")

    nc = _program(causal)
    in_maps = _prep_in_maps(x, W_qkv, W_out)
    res = run_bass_kernel_spmd(nc, in_maps, list(range(NCORE))).results
    # core c owns rows [256c, 256c+256) of each batch
    full = np.empty((S_TOT, DIM), np.float32)
    for c in range(NCORE):
        o = res[c]["out"]
        full[256 * c:256 * c + 256] = o[0:256]
        full[SEQ + 256 * c:SEQ + 256 * c + 256] = o[256:512]
    return np.ascontiguousarray(full.reshape(BATCH, SEQ, DIM), dtype=np.float32)


# revision 49
# speedup vs baseline: 1.0056x; 1.0002x over previous
"""Trainium2 Bass kernel for nn_AcceleratedAttention (GQA transformer block:
fused QKV proj -> causal GQA attention -> out proj), tensor-parallel over
heads on 8 NeuronCores.

Sharding (per core c of 8):
  - q heads {c, c+8, c+16, c+24} and kv head c (all four q heads share kv
    head c, so K/V are computed once per core).
  - Per 512-row s-block: QKV projection, then attention for ALL FOUR heads
    interleaved (QK pairs -> exp on Act -> AV chains pumped with lag), so
    the Activation engine load is spread uniformly and PE stays the
    bottleneck everywhere.
  - Attention runs with scores transposed ([kv, q] tiles).  The AV product
    uses the probability tile as the STATIONARY operand:
    out[q, hd] = pt[kv, q].T @ [V | 1], a 65-column moving matmul at full
    128-row contraction; the ones column delivers the softmax row-sum in
    psum column 64 (normalization = per-partition reciprocal+scale on DVE).
  - ONE merged AllToAll per batch ([2048 rows, 256 chans]; 256-row chunks)
    — the collective cost model charges a flat ~15us per collective, so
    fewer+earlier collectives win.  Core c ends up owning rows
    [256c, 256c+256) of each batch.
  - Epilogue per batch: received [row, chan] blocks are PE-transposed to
    [chan, row]; out-proj = 8 chunks of 16 accumulating matmuls (full
    2048-contraction in one psum bank, no fp16 partial accumulate).
    W_out tiles are preloaded during the last attention blocks.

All matmuls run in fp16 (1 cycle/row on PE; psum fp32).
"""
import sys
from collections import deque

sys.path.insert(0, "/opt/trn_rl_repo")

import numpy as np

DIM = 2048
N_HEADS = 32
N_KV = 8
HD = 64
BATCH = 2
SEQ = 2048
NCORE = 8
S_TOT = BATCH * SEQ          # 4096
S_SLICE = S_TOT // NCORE     # 512
NKC = DIM // 128             # 16 contraction tiles
NSB = S_TOT // 512           # 8 s-blocks for qkv proj
NT = SEQ // 128              # 16 kv tiles per batch
NQB = SEQ // 512             # 4 sq blocks per batch
SCALE = 0.125                # 1/sqrt(64)

_CACHE = {}
TRACE_SIM = False


def _build(causal: bool):
    import concourse.mybir as mybir
    import concourse.tile as tile
    from concourse import bacc
    from concourse.masks import make_identity

    F32 = mybir.dt.float32
    F16 = mybir.dt.float16
    AF = mybir.ActivationFunctionType
    MUL = mybir.AluOpType.mult
    ADD = mybir.AluOpType.add

    nc = bacc.Bacc()
    xT = nc.declare_dram_parameter("xT", [DIM, S_TOT], F16, isOutput=False)
    wq = nc.declare_dram_parameter("wq", [DIM, 384], F16, isOutput=False)
    wo = nc.declare_dram_parameter("wo", [DIM, DIM], F16, isOutput=False)
    ind = nc.declare_dram_parameter("ind", [128, 128], F16, isOutput=False)
    # rows [0,256) = batch0 rows [256c,256c+256), rows [256,512) = batch1
    out = nc.declare_dram_parameter("out", [S_SLICE, DIM], F32, isOutput=True)

    with tile.TileContext(nc, trace_sim=TRACE_SIM) as tc:
        with (
            tc.tile_pool(name="consts", bufs=1) as consts,
            tc.tile_pool(name="qkvp", bufs=1) as qkvp,
            tc.tile_pool(name="vextp", bufs=32) as vextp,
            tc.tile_pool(name="ptp", bufs=17) as ptp,
            tc.tile_pool(name="rsp", bufs=4) as rsp,
            tc.tile_pool(name="ostp", bufs=5) as ostp,
            tc.tile_pool(name="otp", bufs=1) as otp,
            tc.tile_pool(name="aotp", bufs=2) as aotp,
            tc.tile_pool(name="outstp", bufs=3) as outstp,
            tc.tile_pool(name="partp", bufs=8) as partp,
            tc.tile_pool(name="wopA", bufs=2) as wopA,
            tc.tile_pool(name="ps_qkv", bufs=2, space="PSUM") as ps_qkv,
            tc.tile_pool(name="ps_st", bufs=2, space="PSUM") as ps_st,
            tc.tile_pool(name="ps_av", bufs=2, space="PSUM") as ps_av,
            tc.tile_pool(name="dram", bufs=1, space="DRAM") as dram,
        ):
            # ---- constants
            idn = consts.tile([128, 128], F16)
            make_identity(nc, idn[:])
            ind_t = consts.tile([128, 128], F16)
            # loaded on the scalar ring so the first wq chunks own the
            # sync ring from t=0 (ind_t is first read ~10us in)
            nc.scalar.dma_start(ind_t[:], ind[:])

            # Merged AllToAll for batch 0 ([2048 rows, 256 chans]); batch 1
            # split by head pair into two [2048, 128] collectives so the
            # first pair's out-proj half covers the last collective's
            # window.  256-row chunks -> core c gets rows [256c, 256c+256).
            a2a_in = {}
            a2a_out = {}
            a2a_in[0] = dram.tile([SEQ, 256], F16, name="a2ain0")
            a2a_out[0] = dram.tile([SEQ, 256], F16, name="a2aout0")
            for pr in range(2):
                a2a_in[(1, pr)] = dram.tile([SEQ, 128], F16,
                                            name=f"a2ain1p{pr}")
                a2a_out[(1, pr)] = dram.tile([SEQ, 128], F16,
                                             name=f"a2aout1p{pr}")

            xT_r = xT.rearrange("(nk p) s -> p nk s", p=128)
            wo_r = wo.rearrange("(nk p) e -> p nk e", p=128)

            # deferred AV chains, pumped with ~4-item lag behind exp
            pend_a = deque()

            def pump(q, drain_to=0):
                while len(q) > drain_to:
                    q.popleft()()

            # ---- qkv tiles
            qkv_t = {(m, sb): qkvp.tile([128, 512], F16, tag=f"qkv{m}_{sb}",
                                        name=f"qkv{m}_{sb}")
                     for m in range(3) for sb in range(NSB)}
            # K replicated at partitions 64:128 so odd heads' Q (upper
            # partition half) sees a matching base partition.
            kv2_t = {sb: qkvp.tile([128, 512], F16, tag=f"kv2_{sb}",
                                   name=f"kv2_{sb}")
                     for sb in range(NSB)}

            def q_ap(h, b, sqb, c0, c1):
                t = qkv_t[(h // 2, b * NQB + sqb)]
                base = 64 * (h % 2)
                return t[base:base + 64, c0:c1]

            def k_ap(h, b, j):
                sb, off = divmod(j, 4)
                if h % 2 == 0:
                    return qkv_t[(2, b * NQB + sb)][0:64,
                                                    off * 128:(off + 1) * 128]
                return kv2_t[b * NQB + sb][64:128, off * 128:(off + 1) * 128]

            vexts = {}

            # ---- phase 1 (QKV projection), one 512-row s-block at a time
            with (
                tc.tile_pool(name="wqp", bufs=1) as wqp,
                tc.tile_pool(name="xtp", bufs=2) as xtp,
            ):
                wq_t = wqp.tile([128, NKC, 384], F16)
                wq_r = wq.rearrange("(nk p) m -> p nk m", p=128)
                nc.sync.dma_start(wq_t[:, 0:1], wq_r[:, 0:1])
                nc.sync.dma_start(wq_t[:, 1:3], wq_r[:, 1:3])
                nc.sync.dma_start(wq_t[:, 3:8], wq_r[:, 3:8])
                nc.sync.dma_start(wq_t[:, 8:NKC], wq_r[:, 8:NKC])

                # phase-1 work for a block is split into single-op closures
                # so it can be drip-fed between attention pairs of the
                # PREVIOUS block (fills PE during Act-bound stretches and
                # keeps the p-state hot).
                pend_p1 = deque()

                def phase1_closures(sb):
                    xt = xtp.tile([128, NKC, 512], F16, tag="xt",
                                  name=f"xt{sb}")
                    sl = slice(sb * 512, (sb + 1) * 512)
                    if sb == 0:
                        # split across two idle DMA rings at startup
                        nc.gpsimd.dma_start(xt[:, 0:1], xT_r[:, 0:1, sl])
                        nc.scalar.dma_start(xt[:, 8:10], xT_r[:, 8:10, sl])
                        nc.gpsimd.dma_start(xt[:, 1:2], xT_r[:, 1:2, sl])
                        nc.gpsimd.dma_start(xt[:, 2:4], xT_r[:, 2:4, sl])
                        nc.scalar.dma_start(xt[:, 10:NKC], xT_r[:, 10:NKC, sl])
                        nc.gpsimd.dma_start(xt[:, 4:8], xT_r[:, 4:8, sl])
                    else:
                        nc.gpsimd.dma_start(xt[:], xT_r[:, :, sl])
                    ops = []
                    for m in (2, 0, 1):  # kv first: feeds vext early
                        cell = {}

                        def mk_mm(m, k, cell):
                            def mm():
                                if k == 0:
                                    cell["acc"] = ps_qkv.tile(
                                        [128, 512], F32, tag="qkv",
                                        name=f"qkvacc{sb}_{m}")
                                nc.tensor.matmul(
                                    cell["acc"][:],
                                    wq_t[:, k, m * 128:(m + 1) * 128],
                                    xt[:, k, :],
                                    start=(k == 0),
                                    stop=(k == NKC - 1),
                                )
                            return mm

                        for k in range(NKC):
                            ops.append(mk_mm(m, k, cell))

                        def mk_copy(m, cell):
                            def cp():
                                nc.vector.tensor_copy(
                                    qkv_t[(m, sb)][:], cell["acc"][:])
                                if m == 2:
                                    nc.sync.dma_start(
                                        kv2_t[sb][64:128, :],
                                        qkv_t[(2, sb)][0:64, :])
                            return cp

                        ops.append(mk_copy(m, cell))

                    b, sqb = divmod(sb, NQB)
                    for jo in range(4):
                        j = sqb * 4 + jo
                        vexts[(b, j)] = vextp.tile(
                            [128, 65], F16, tag="vext", name=f"vx_{b}_{j}")

                        def mk_vext(jo, j):
                            def vext():
                                vx = vexts[(b, j)]
                                tp = ps_st.tile([128, 64], F16, tag="st",
                                                name=f"vt_{b}_{j}")
                                nc.tensor.transpose(
                                    tp[:],
                                    qkv_t[(2, sb)][64:128,
                                                   jo * 128:(jo + 1) * 128],
                                    idn[64:128, 64:128],
                                )
                                nc.vector.tensor_copy(vx[:, 0:64], tp[:])
                                nc.vector.memset(vx[:, 64:65], 1.0)
                            return vext

                        ops.append(mk_vext(jo, j))
                    return ops

                # ---- attention machinery
                ost_tiles = {}
                ost_count = {}

                def get_ost(h, b):
                    if (h, b) not in ost_tiles:
                        ost_tiles[(h, b)] = ostp.tile(
                            [128, 16, HD], F16, tag="ost", name=f"ost{h}_{b}")
                        ost_count[(h, b)] = 0
                    return ost_tiles[(h, b)]

                def finish_ost(h, b, half=None):
                    # half=0: rows qt 0:8 eagerly; half=1: qt 8:16 (tile
                    # freed).  Splitting lets the collective start sooner
                    # after the last chain.
                    if half == 0:
                        ost = ost_tiles[(h, b)]
                    else:
                        ost = ost_tiles.pop((h, b))
                    buf = a2a_in[0] if b == 0 else a2a_in[(1, h // 2)]
                    co = 64 * h if b == 0 else 64 * (h % 2)
                    dst = buf.rearrange("(j q2 p) c -> p j q2 c",
                                        p=128, q2=2)
                    if half == 0:
                        nc.sync.dma_start(dst[:, 0:4, :, co:co + 64],
                                          ost[:, 0:8])
                    else:
                        nc.sync.dma_start(dst[:, 4:8, :, co:co + 64],
                                          ost[:, 8:16])

                def make_chain(h, b, sqb, qco, pts, offs):
                    qt = 4 * sqb + qco
                    jhi = qt if causal else NT - 1

                    def chain():
                        av = ps_av.tile([128, 65], F32, tag="av",
                                        name=f"av{h}_{b}_{qt}")
                        for j in range(jhi + 1):
                            c0 = qco * 128 - offs[j]
                            nc.tensor.matmul(
                                av[:],
                                pts[j][:, c0:c0 + 128],
                                vexts[(b, j)][:],
                                start=(j == 0),
                                stop=(j == jhi),
                                skip_group_check=True,
                            )
                        rsr = rsp.tile([128, 1], F32, tag="rsr",
                                       name=f"rsr{h}_{b}_{qt}")
                        nc.vector.reciprocal_approx_fast(rsr[:], av[:, 64:65])
                        ost = get_ost(h, b)
                        nc.vector.tensor_scalar_mul(
                            ost[:, qt, :], av[:, 0:64], rsr[:])
                        ost_count[(h, b)] += 1
                        if ost_count[(h, b)] == 8:
                            finish_ost(h, b, half=0)
                        elif ost_count[(h, b)] == 16:
                            finish_ost(h, b, half=1)

                    return chain

                def attn_block(h, b, sqb, tmod=2, last=False):
                    jmax = 4 * sqb + 3 if causal else NT - 1
                    ndiag0 = 4 * sqb if causal else jmax + 1
                    pts = {}
                    offs = {}
                    get_ost(h, b)
                    npair = (jmax + 1) // 2

                    def qk_pair(gi):
                        stp = ps_st.tile([128, 2, 512], F32, tag="st",
                                         name=f"st{h}_{b}_{sqb}_{gi}")
                        for idx in range(2):
                            j = 2 * gi + idx
                            off = (max(0, j * 128 - sqb * 512)
                                   if causal else 0)
                            n = 512 - off
                            nc.tensor.matmul(
                                stp[:, idx, :n],
                                k_ap(h, b, j),
                                q_ap(h, b, sqb, off, 512),
                                start=True,
                                stop=True,
                            )
                            offs[j] = off
                        return stp

                    def exp_pair(gi, stp):
                        j0 = 2 * gi
                        pt2 = ptp.tile([128, 2, 512], F16, tag="pt",
                                       name=f"pt{h}_{b}_{sqb}_{gi}")
                        if j0 + 1 < ndiag0:
                            # both halves unmasked: one fused activation
                            nc.scalar.activation(
                                pt2[:], stp[:], AF.Exp, scale=SCALE)
                        else:
                            for idx in range(2):
                                j = j0 + idx
                                n = 512 - offs[j]
                                nc.scalar.activation(
                                    pt2[:, idx, :n], stp[:, idx, :n],
                                    AF.Exp, scale=SCALE)
                                if causal and j >= ndiag0:
                                    nc.vector.tensor_tensor(
                                        pt2[:, idx, 0:128],
                                        pt2[:, idx, 0:128], ind_t[:], MUL)
                        pts[j0] = pt2[:, 0, :]
                        pts[j0 + 1] = pt2[:, 1, :]

                    stps = {0: qk_pair(0)}
                    for gi in range(npair):
                        if gi + 1 < npair:
                            stps[gi + 1] = qk_pair(gi + 1)
                        exp_pair(gi, stps.pop(gi))
                        for idx in range(2):
                            j = 2 * gi + idx
                            pump(pend_a, 4)
                            for _ in range(2):
                                if pend_p1:
                                    pend_p1.popleft()()
                            if (not pend_p1 and pend_tail
                                    and j % tmod == 0
                                    and not (last and gi >= npair - 3)):
                                pend_tail.popleft()()
                            if causal and j >= ndiag0:
                                pend_a.append(
                                    make_chain(h, b, sqb, j - ndiag0,
                                               pts, offs))
                    if not causal:
                        for qco in range(4):
                            pend_a.append(
                                make_chain(h, b, sqb, qco, pts, offs))
                        pump(pend_a, 4)

                def emit_a2a(key):
                    nc.gpsimd.collective_compute(
                        "AllToAll",
                        mybir.AluOpType.bypass,
                        replica_groups=[list(range(NCORE))],
                        ins=[a2a_in[key].opt()],
                        outs=[a2a_out[key].opt()],
                    )

                # ================= emission =================
                wot = {}

                def load_wo(eb, pool, queue):
                    w = pool.tile([128, NKC, 512], F16, tag="wo",
                                  name=f"wo{eb}")
                    queue.dma_start(
                        w[:], wo_r[:, :, eb * 512:(eb + 1) * 512])
                    wot[eb] = w

                ot_tiles = {}
                pend_tail = deque()

                def emit_ot(b):
                    ot = otp.tile([128, 2, 8, 256], F16, tag="ot",
                                  name=f"ot{b}")
                    src = a2a_out[b].rearrange("(j q2 p) c -> p q2 j c",
                                               p=128, q2=2)
                    nc.gpsimd.dma_start(ot[:, 0], src[:, 0])
                    nc.gpsimd.dma_start(ot[:, 1], src[:, 1])
                    aoT = aotp.tile([128, 2, 16, 128], F16, tag="aot",
                                    name=f"aoT{b}")
                    ot_tiles[b] = (ot, aoT)

                def emit_oth(pr):
                    # batch-1 pair buffer: chans [0:128] of kt = 2j + pr
                    ot = otp.tile([128, 2, 8, 128], F16, tag="oth",
                                  name=f"ot1p{pr}")
                    src = a2a_out[(1, pr)].rearrange(
                        "(j q2 p) c -> p q2 j c", p=128, q2=2)
                    nc.gpsimd.dma_start(ot[:, 0], src[:, 0])
                    nc.scalar.dma_start(ot[:, 1], src[:, 1])
                    if 1 not in ot_tiles:
                        # [p, ss, pair, j, c]: kt = 2j + pair
                        aoT = aotp.tile([128, 2, 2, 8, 128], F16, tag="aot",
                                        name="aoT1")
                        ot_tiles[1] = (None, aoT)
                    return ot

                def tp4(b, ss, gg):
                    # 4 PE transposes into one psum tile + one DVE copy
                    def f():
                        ot, aoT = ot_tiles[b]
                        tpt = ps_st.tile([128, 4, 128], F16, tag="st",
                                         name=f"tp{b}_{ss}_{gg}")
                        for i in range(4):
                            kt = gg * 4 + i
                            j, ch = divmod(kt, 2)
                            nc.tensor.transpose(
                                tpt[:, i, :],
                                ot[:, ss, j, ch * 128:(ch + 1) * 128],
                                idn[:],
                            )
                        nc.vector.tensor_copy(
                            aoT[:, ss, gg * 4:gg * 4 + 4, :], tpt[:])
                    return f

                def tp4h(oth, pr, ss, jg):
                    # 4 transposes of b1 pair pr, source j in [4jg, 4jg+4)
                    def f():
                        _, aoT = ot_tiles[1]
                        tpt = ps_st.tile([128, 4, 128], F16, tag="st",
                                         name=f"tph{pr}_{ss}_{jg}")
                        for i in range(4):
                            j = jg * 4 + i
                            nc.tensor.transpose(
                                tpt[:, i, :], oth[:, ss, j, :], idn[:])
                        nc.vector.tensor_copy(
                            aoT[:, ss, pr, jg * 4:jg * 4 + 4, :], tpt[:])
                    return f

                parts = {}

                def half_chunk(eb, ss, pr):
                    # b1 out-proj over kt = pr, pr+2, ..., pr+14.  Pair 0
                    # parks the partial in fp16 SBUF; pair 1 adds it back.
                    def f():
                        _, aoT = ot_tiles[1]
                        ops = ps_qkv.tile([128, 512], F32, tag="qkv",
                                          name=f"opsh{eb}_{ss}_{pr}")
                        for jj in range(8):
                            nc.tensor.matmul(
                                ops[:],
                                aoT[:, ss, pr, jj, :],
                                wot[eb][:, 2 * jj + pr, :],
                                start=(jj == 0),
                                stop=(jj == 7),
                            )
                        if pr == 0:
                            part = partp.tile([128, 512], F16, tag="part",
                                              name=f"part{eb}_{ss}")
                            nc.vector.tensor_copy(part[:], ops[:])
                            parts[(eb, ss)] = part
                        else:
                            og = outstp.tile([128, 512], F32, tag="out",
                                             name=f"ogh{eb}_{ss}")
                            nc.vector.tensor_tensor(
                                og[:], parts[(eb, ss)][:], ops[:], ADD)
                            nc.sync.dma_start(
                                out[256 + ss * 128:256 + ss * 128 + 128,
                                    eb * 512:(eb + 1) * 512],
                                og[:],
                            )
                    return f

                def chunk_quarters(b, eb, ss):
                    # out-proj chunk split into 4 pumpable quarters
                    cell = {}

                    def mk_q(q):
                        def f():
                            _, aoT = ot_tiles[b]
                            if q == 0:
                                cell["ops"] = ps_qkv.tile(
                                    [128, 512], F32, tag="qkv",
                                    name=f"ops{b}_{eb}_{ss}")
                            for k in range(4 * q, 4 * q + 4):
                                nc.tensor.matmul(
                                    cell["ops"][:],
                                    aoT[:, ss, k, :],
                                    wot[eb][:, k, :],
                                    start=(k == 0),
                                    stop=(k == NKC - 1),
                                )
                            if q == 3:
                                og = outstp.tile([128, 512], F32, tag="out",
                                                 name=f"og{b}_{eb}_{ss}")
                                nc.vector.tensor_copy(og[:], cell["ops"][:])
                                nc.sync.dma_start(
                                    out[b * 256 + ss * 128:
                                        b * 256 + ss * 128 + 128,
                                        eb * 512:(eb + 1) * 512],
                                    og[:],
                                )
                        return f

                    return [mk_q(q) for q in range(4)]

                for sb in range(NSB):
                    if sb == 0:
                        for op in phase1_closures(0):
                            op()
                    pump(pend_p1)  # leftover phase-1 ops of this block
                    if sb + 1 < NSB:
                        pend_p1.extend(phase1_closures(sb + 1))
                    b, sqb = divmod(sb, NQB)
                    for h in (range(4) if b == 0 else (0, 1, 2)):
                        attn_block(h, b, sqb)
                        if sb == NSB - 1 and h == 1:
                            # heads 0,1 of batch 1 complete -> pair-0
                            # collective overlaps h2/h3 tail attention
                            pump(pend_a)
                            emit_a2a((1, 0))
                    if sb == NQB - 1:
                        # batch 0 attention fully emitted; chains drain in
                        # the pump lag -> collective as soon as osts land
                        pump(pend_a)
                        emit_a2a(0)
                        emit_ot(0)
                        load_wo(0, wopA, nc.sync)
                        load_wo(1, wopA, nc.sync)
                        # batch-0 epilogue work for the preloaded k-halves
                        # becomes PE filler during batch-1 attention
                        for ss in range(2):
                            for gg in range(4):
                                pend_tail.append(tp4(0, ss, gg))
                            for eb in (0, 1):
                                pend_tail.extend(chunk_quarters(0, eb, ss))
            # ---- batch-1 head 3 + epilogue.  Pair-0's collective runs
            # during this attention; its out-proj half covers pair-1's.
            # Unpumped b0 quarters become the reserve for the final window.
            with tc.tile_pool(name="wopB", bufs=2) as wopB:
                load_wo(2, wopB, nc.sync)
                load_wo(3, wopB, nc.sync)
                for ss in range(2):
                    for eb in (2, 3):
                        pend_tail.extend(chunk_quarters(0, eb, ss))
                # throttled pumping: keep most of the b0 reserve for the
                # pair-1 collective window; none in the last block so the
                # final chains' normalize + ost DMA aren't queued behind
                # tail copies
                for sqb in range(NQB):
                    attn_block(3, 1, sqb,
                               tmod=3 if sqb < NQB - 1 else 12,
                               last=(sqb == NQB - 1))
                pump(pend_a)
                emit_a2a((1, 1))

                # pair-0 epilogue half (fills pair-1's collective window),
                # then the held-back b0 chunks cover the ot1p1 latency
                ot_p0 = emit_oth(0)
                for ss in range(2):
                    for jg in range(2):
                        tp4h(ot_p0, 0, ss, jg)()
                for ss in range(2):
                    for eb in range(4):
                        half_chunk(eb, ss, 0)()
                pump(pend_tail)
                # PE warm-up filler: keep the tensor engine busy (and the
                # p-state at full clock) across the pair-1 collective's
                # landing latency; results are never read
                warm = ps_qkv.tile([128, 128], F16, tag="qkv", name="warm")
                for _ in range(98):
                    nc.tensor.transpose(warm[:], idn[:], idn[:])
                # pair-1 epilogue half
                ot_p1 = emit_oth(1)
                for ss in range(2):
                    for jg in range(2):
                        tp4h(ot_p1, 1, ss, jg)()
                for ss in range(2):
                    for eb in range(4):
                        half_chunk(eb, ss, 1)()

    nc.compile()
    return nc


def _program(causal: bool):
    if causal not in _CACHE:
        _CACHE[causal] = _build(causal)
    return _CACHE[causal]


def _prep_in_maps(x, W_qkv, W_out):
    xT = x.reshape(S_TOT, DIM).T.astype(np.float16)  # [c, s] contiguous
    ind_np = (np.arange(128)[None, :] >= np.arange(128)[:, None]).astype(np.float16)

    # W_out cols permuted to the received-block channel order: block j
    # (256 chans) carries core j's heads {j, j+8, j+16, j+24}.
    wt = W_out.T.reshape(N_HEADS, HD, DIM)
    woT = np.ascontiguousarray(np.concatenate(
        [wt[[j, j + 8, j + 16, j + 24]].reshape(4 * HD, DIM)
         for j in range(NCORE)]
    )).astype(np.float16)

    in_maps = []
    for c in range(NCORE):
        rows = np.concatenate(
            [np.arange(h * HD, (h + 1) * HD) for h in (c, c + 8, c + 16, c + 24)]
            + [np.arange(2048 + c * HD, 2048 + (c + 1) * HD),
               np.arange(2560 + c * HD, 2560 + (c + 1) * HD)]
        )
        wq_shard = np.ascontiguousarray(W_qkv[rows].T).astype(np.float16)
        in_maps.append({"xT": xT, "wq": wq_shard, "wo": woT, "ind": ind_np})
    return in_maps


def kernel(**inputs) -> np.ndarray:
    from concourse.bass_utils import run_bass_kernel_spmd

    x = np.asarray(inputs["x"], np.float32)
    W_qkv = np.asarray(inputs["W_qkv"], np.float32)
    W_out = np.asarray(inputs["W_out"], np.float32)
    mask = np.asarray(inputs["mask"], np.float32)

    m2 = mask.reshape(SEQ, SEQ)
    iu = np.triu_indices(SEQ, 1)
    causal = bool((m2[iu] <= -1e8).all()) and bool((np.tril(m2) == 0.0).all())
    dense = bool((m2 == 0.0).all())
    if not (causal or dense):
        raise NotImplementedError("only causal or all-zero masks supported")

    nc = _program(causal)
    in_maps = _prep_in_maps(x, W_qkv, W_out)
    res = run_bass_kernel_spmd(nc, in_maps, list(range(NCORE))).results
    # core c owns rows [256c, 256c+256) of each batch
    full = np.empty((S_TOT, DIM), np.float32)
    for c in range(NCORE):
        o = res[c]["out"]
        full[256 * c:256 * c + 256] = o[0:256]
        full[SEQ + 256 * c:SEQ + 256 * c + 256] = o[256:512]
    return np.ascontiguousarray(full.reshape(BATCH, SEQ, DIM), dtype=np.float32)
